# revision 1
# baseline (speedup 1.0000x reference)
"""Trainium2 Bass kernel for FASTMultiHeadAttention (fastmax, Taylor-2 softmax approx
with relative positional embeddings, optional causal mask).

B=1, H=8, N=2048, D=64. One head per NeuronCore (8 cores).

Math per head (q,k,v: [N,D], rpe: [2N-1, D]):
    s[i,j]  = q_i.k_j + q_i.rpe[i-j+N-1]
    w       = 1 + s + s^2/2      (causal-masked if mask)
    out_i   = sum_j w[i,j] v_j / sum_j w[i,j]

Device algorithm (per head):
    w = ((s+1)^2 + 1)/2 on valid entries, so with t = (s+1)^2 (t=0 on masked):
      numer_i = 0.5*(sum_j t_ij v_j + sum_{j<=i} v_j)
      denom_i = 0.5*(sum_j t_ij + (i+1))
      out_i   = (sum_j t v + vcum_i) / (sum_j t + (i+1))
    - content scores:  PE matmul (f32r)  S = qT.T @ kT        [i-part, j-free]
    - rpe scores:      PE matmul QR = qT.T @ rpe_revT windows, then a diagonal
      SBUF->SBUF DMA gather R[a,j] = QR[a, 127 - a + j]  (per-partition shift)
    - s1 = (S + 1) + R   via scalar_tensor_tensor (DVE)
    - causal mask: affine_select fills s1 with 0 where j > i (GPSIMD)
    - t = s1 * s1 with accum_out giving row sums (denominator)
    - W^T via PE transposes (bf16) -> O = sum_j t v via PE matmul with V stationary
"""

import sys
import os
import numpy as np

for _p in ("/opt/trn_rl_repo", "/root/.axon_site/_ro/trn_rl_repo"):
    if os.path.isdir(_p) and _p not in sys.path:
        sys.path.insert(0, _p)

B, H, N, D = 1, 8, 2048, 64
NT = N // 128            # 16 i-tiles of 128 rows
JT = 512                 # j-tile width
NJC = N // 128           # 16 j-chunks of 128 (for transposes / O matmul)

_CACHE = {}

# engine-assignment tuning (fractions routed to the listed engine)
TUNE = {
    "sq_act_frac": 1.0,     # square-evac ops on ACT (else DVE stt)
    "qr_act_frac": 0.5,     # QR psum->sbuf copies on ACT (else DVE)
    "stt_pool_frac": 0.0,   # s1 chunks via ACT-evac + Pool TT-add (else DVE stt)
    "loads_swdge": False,   # input loads via gpsimd (SWDGE) instead of sync
    "gather_act_frac": 0.0, # gathers issued from ACT ring (else SP)
    "stt_chain_frac": 0.0,  # s1 chunks via ACT copy(+1) + DVE bf16 TT-add
}


def _build_program(causal: bool, reps: int = 1):
    import concourse.bass as bass
    from concourse import bacc
    import concourse.mybir as mybir
    from concourse.tile import TileContext
    from concourse.masks import make_identity

    fp32 = mybir.dt.float32
    f32r = mybir.dt.float32r
    bf16 = mybir.dt.bfloat16
    AT = mybir.ActivationFunctionType
    OP = mybir.AluOpType

    RPW = 2560 if causal else 4608   # rpe_revT padded width
    QRW = 2560                       # QR buffer width (per i-tile u-window)

    nc = bacc.Bacc("TRN2", target_bir_lowering=False, debug=False)

    qT_d = nc.dram_tensor("qT", [64, N], fp32, kind="ExternalInput")
    kT_d = nc.dram_tensor("kT", [64, N], fp32, kind="ExternalInput")
    v_d = nc.dram_tensor("vr", [128, NJC * 66], fp32, kind="ExternalInput")
    vcum_d = nc.dram_tensor("vcum", [128, NT * 64], fp32, kind="ExternalInput")
    rpe_d = nc.dram_tensor("rpeT", [64, RPW], fp32, kind="ExternalInput")
    iota_d = nc.dram_tensor("iota", [128, NT], fp32, kind="ExternalInput")
    o_d = nc.dram_tensor("o", [N, 64], fp32, kind="ExternalOutput")

    with TileContext(nc) as tc:
        with (
            tc.tile_pool(name="persist", bufs=1) as pp,
            tc.tile_pool(name="qr", bufs=(4 if causal else 3)) as qrp,
            tc.tile_pool(name="rows", bufs=(6 if causal else 4)) as rp,
            tc.tile_pool(name="small", bufs=2) as sp,
        ):
            class Frac:
                # weighted deterministic router: pick() True with rate `frac`
                def __init__(self, frac):
                    self.f = frac
                    self.acc = 0.0
                def pick(self):
                    self.acc += self.f
                    if self.acc >= 0.999:
                        self.acc -= 1.0
                        return True
                    return False

            qr_r = Frac(TUNE["qr_act_frac"])
            sq_r = Frac(TUNE["sq_act_frac"])
            sttp_r = Frac(TUNE["stt_pool_frac"])
            sttc_r = Frac(TUNE["stt_chain_frac"])
            ga_r = Frac(TUNE["gather_act_frac"])

            def copy_alt(dst, srcap):
                if qr_r.pick():
                    nc.scalar.activation(dst, srcap, AT.Copy, bias=0.0, scale=1.0)
                else:
                    nc.vector.tensor_copy(dst, srcap)

            # ---- persistent loads (q/k/rpe duplicated into both partition
            # halves so K=64 matmuls can pack two PE row-groups) ----
            qT_f = pp.tile([128, N], fp32, name="qT_f")
            kT_f = pp.tile([128, N], fp32, name="kT_f")
            rpe_f = pp.tile([128, RPW], fp32, name="rpe_f")
            v_f = pp.tile([128, NJC * 66], fp32, name="v_f")
            vcum_s = pp.tile([128, NT * 64], fp32, name="vcum_s")
            iota_s = pp.tile([128, NT], fp32, name="iota_s")
            ld = nc.gpsimd if TUNE["loads_swdge"] else nc.sync
            for half, eng in ((0, nc.sync), (64, nc.scalar)):
                eng.dma_start(out=qT_f[half:half + 64, :], in_=qT_d.ap())
                eng.dma_start(out=kT_f[half:half + 64, :], in_=kT_d.ap())
                eng.dma_start(out=rpe_f[half:half + 64, :], in_=rpe_d.ap())
            ld.dma_start(out=v_f[:], in_=v_d.ap())
            ld.dma_start(out=vcum_s[:], in_=vcum_d.ap())
            ld.dma_start(out=iota_s[:], in_=iota_d.ap())

            # round to f32r for PE (required by BIR verifier)
            qT_r = pp.tile([128, N], f32r, name="qT_r")
            kT_r = pp.tile([128, N], f32r, name="kT_r")
            rpe_r = pp.tile([128, RPW], f32r, name="rpe_r")
            nc.vector.tensor_copy(qT_r[:], qT_f[:])
            nc.vector.tensor_copy(kT_r[:], kT_f[:])
            nc.vector.tensor_copy(rpe_r[:], rpe_f[:])

            v_bf = pp.tile([128, NJC * 66], bf16, name="v_bf")
            nc.vector.tensor_copy(v_bf[:], v_f[:])

            ident = pp.tile([128, 128], bf16, name="ident")
            make_identity(nc, ident[:])
            ident66_f = pp.tile([66, 66], fp32, name="ident66_f")
            make_identity(nc, ident66_f[:])
            ident66 = pp.tile([66, 66], f32r, name="ident66")
            nc.vector.tensor_copy(ident66[:], ident66_f[:])

            # W^T storage, triangular-packed by groups of 4 j-chunks when causal:
            # group g0 stores only i >= 128*g0 (width Wg = N - 128*g0).
            def wt_imin(jc):
                return 128 * (4 * (jc // 4)) if causal else 0

            def wt_w(jc):
                return N - wt_imin(jc)

            _wt_base = {}
            _off = 0
            for _jc in range(NJC):
                _wt_base[_jc] = _off
                _off += wt_w(_jc)
            WTW = _off
            wt_all = pp.tile([128, WTW], bf16, name="wt_all")

            # accumulator [128, NT*65]: per tile 64 numerator cols + denom col
            O_all = pp.tile([128, NT * 65], fp32, name="O_all")
            out_s = pp.tile([128, NT * 64], fp32, name="out_s")

            for _rep in range(reps):
              with (
                  tc.tile_pool(name="mm_ps", bufs=3, space="PSUM") as mmp,
                  tc.tile_pool(name="tr_ps", bufs=2, space="PSUM") as trp,
              ):
                live = {}
                _geng = [0]

                def gather_dma(dst, srcap):
                    eng = nc.scalar if ga_r.pick() else nc.sync
                    eng.dma_start(out=dst, in_=srcap)

                def mm_packed(out_ps, t, src_r, c0, mw):
                    # K=64 matmul; row-group alternates per i-tile so stageA(t+1)
                    # and stageB(t) matmuls pack into different PE row halves
                    # without thrashing the weight registers within a tile.
                    i0 = 128 * t
                    g = t & 1
                    nc.tensor.matmul(out_ps, qT_r[64 * g:64 * g + 64, i0:i0 + 128],
                                     src_r[64 * g:64 * g + 64, c0:c0 + mw],
                                     start=True, stop=True, tile_position=(64 * g, 0))

                def stageA(t):
                    # rpe projection QR + diagonal gather of R
                    i0 = 128 * t
                    j_max = i0 + 128 if causal else N
                    u_min = (N - 1) - i0 - 127
                    qr_w = 127 + j_max
                    qrbuf = qrp.tile([128, QRW], bf16, name="qrbuf")
                    for b0 in range(0, qr_w, 1024):
                        bw = min(1024, qr_w - b0)
                        mm_ps = mmp.tile([128, 1024], fp32, name="mm_ps")
                        for h0 in range(0, bw, 512):
                            hw = min(512, bw - h0)
                            mw = max(256, (hw + 1) & ~1)  # f32r ISA: even, >= 256
                            mm_packed(mm_ps[:, h0:h0 + mw], t, rpe_r, u_min + b0 + h0, mw)
                        copy_alt(qrbuf[:, b0:b0 + bw], mm_ps[:, 0:bw])
                    # diagonal gather R[a, j] = qrbuf[a, 127 - a + j]
                    R_row = rp.tile([128, N], bf16, name="R_row")
                    diag = bass.AP(qrbuf[:].tensor, qrbuf[:].offset + 127,
                                   [[QRW - 1, 128], [1, j_max]])
                    gather_dma(R_row[:, 0:j_max], diag)
                    live[t] = (qrbuf, R_row)

                def stageB(t):
                    # content scores + s1 + mask
                    i0 = 128 * t
                    j_max = i0 + 128 if causal else N
                    _, R_row = live[t]
                    s1_row = rp.tile([128, N], bf16, name="s1_row")
                    for jb in range(0, j_max, 1024):
                        cw = min(1024, j_max - jb)
                        mm_ps = mmp.tile([128, 1024], fp32, name="mm_ps")
                        for h0 in range(0, cw, 512):
                            hw = min(512, cw - h0)
                            mw = max(256, (hw + 1) & ~1)
                            mm_packed(mm_ps[:, h0:h0 + mw], t, kT_r, jb + h0, mw)
                        if sttc_r.pick():
                            cs_bf = rp.tile([128, 1024], bf16, name="cs_bf", tag="cs_bf")
                            nc.scalar.activation(cs_bf[:, 0:cw], mm_ps[:, 0:cw],
                                                 AT.Copy, bias=1.0, scale=1.0)
                            nc.vector.tensor_tensor(
                                out=s1_row[:, jb:jb + cw], in0=cs_bf[:, 0:cw],
                                in1=R_row[:, jb:jb + cw], op=OP.add)
                        elif sttp_r.pick():
                            cs_bf = rp.tile([128, 1024], bf16, name="cs_bf", tag="cs_bf")
                            nc.scalar.activation(cs_bf[:, 0:cw], mm_ps[:, 0:cw],
                                                 AT.Copy, bias=1.0, scale=1.0)
                            nc.gpsimd.tensor_tensor(
                                out=s1_row[:, jb:jb + cw], in0=cs_bf[:, 0:cw],
                                in1=R_row[:, jb:jb + cw], op=OP.add)
                        else:
                            nc.vector.scalar_tensor_tensor(
                                out=s1_row[:, jb:jb + cw], in0=mm_ps[:, 0:cw], scalar=1.0,
                                in1=R_row[:, jb:jb + cw], op0=OP.add, op1=OP.add)
                    s1_diag = None
                    if causal:
                        # masked diagonal chunk goes to its own tile so the mask
                        # doesn't gate the other chunks' transposes
                        s1_diag = rp.tile([128, 128], bf16, name="s1_diag", tag="s1_diag")
                        nc.gpsimd.affine_select(
                            out=s1_diag[:], in_=s1_row[:, i0:i0 + 128],
                            compare_op=OP.is_ge, fill=0.0,
                            base=0, channel_multiplier=1, pattern=[[-1, 128]])
                    live[t] = (s1_diag, s1_row)

                def stageC(t):
                    # transpose s1 chunks, square during PSUM evacuation -> wt_all = t^T
                    i0 = 128 * t
                    j_max = i0 + 128 if causal else N
                    s1_diag, s1_row = live.pop(t)
                    njc = (j_max + 127) // 128
                    for g0 in range(0, njc, 4):
                        gn = min(4, njc - g0)
                        tr_ps = trp.tile([128, 512], bf16, name="tr_ps")
                        for g in range(gn):
                            jc = g0 + g
                            src_chunk = (s1_diag[:] if (causal and jc == t)
                                         else s1_row[:, 128 * jc:128 * (jc + 1)])
                            nc.tensor.transpose(tr_ps[:, 128 * g:128 * (g + 1)],
                                                src_chunk, ident[:])
                        dst = bass.AP(wt_all[:].tensor,
                                      wt_all[:].offset + _wt_base[g0] + (i0 - wt_imin(g0)),
                                      [[WTW, 128], [wt_w(g0), gn], [1, 128]])
                        srcap = tr_ps[:, 0:128 * gn].rearrange("p (g c) -> p g c", g=gn)
                        if sq_r.pick():
                            nc.scalar.activation(dst, srcap, AT.Square, bias=0.0, scale=1.0)
                        else:
                            nc.vector.scalar_tensor_tensor(
                                out=dst, in0=srcap, scalar=1.0, in1=srcap,
                                op0=OP.mult, op1=OP.mult)

                def stageO(s, op_, obp):
                    # output matmuls for i-slab s, back-transpose, normalize, store
                    i0s = 512 * s
                    o_ps = op_.tile([66, 512], fp32, name="o_ps")
                    jc_hi = 4 * s + 4 if causal else NJC
                    started = False
                    for jc in range(jc_hi):
                        lo = max(i0s, 128 * jc) if causal else i0s
                        w = 512 * s + 512 - lo
                        rhs = bass.AP(wt_all[:].tensor,
                                      wt_all[:].offset + _wt_base[jc] + (lo - wt_imin(jc)),
                                      [[WTW, 128], [1, w]])
                        nc.tensor.matmul(o_ps[:, lo - i0s:512], v_bf[:, 66 * jc:66 * (jc + 1)], rhs,
                                         start=(not started), stop=(jc == jc_hi - 1))
                        started = True
                    oT_s = sp.tile([66, 512], f32r, name="oT_s")
                    nc.vector.tensor_copy(oT_s[:], o_ps[:])
                    for g in range(4):
                        tt = s * 4 + g
                        ob_ps = obp.tile([128, 66], f32r, name="ob_ps")
                        nc.tensor.transpose(ob_ps[:], oT_s[:, 128 * g:128 * (g + 1)],
                                            ident66[:])
                        copy_alt(O_all[:, 65 * tt:65 * (tt + 1)], ob_ps[:, 0:65])
                    # normalize this slab: denom col 64 + iota -> recip; out = (num+vcum)*recip
                    t0, t1 = 4 * s, 4 * s + 4
                    dtot = sp.tile([128, 4], fp32, name="dtot", tag="dtot")
                    dcol = bass.AP(O_all[:].tensor, O_all[:].offset + 65 * t0 + 64,
                                   [[NT * 65, 128], [65, 4]])
                    nc.vector.tensor_tensor(out=dtot[:], in0=dcol, in1=iota_s[:, t0:t1], op=OP.add)
                    recip = sp.tile([128, 4], fp32, name="recip", tag="recip")
                    nc.vector.reciprocal(recip[:], dtot[:])
                    onum = bass.AP(O_all[:].tensor, O_all[:].offset + 65 * t0,
                                   [[NT * 65, 128], [65, 4], [1, 64]])
                    osl = out_s[:, 64 * t0:64 * t1].rearrange("p (t d) -> p t d", d=64)
                    nc.vector.tensor_tensor(
                        out=osl, in0=onum,
                        in1=vcum_s[:, 64 * t0:64 * t1].rearrange("p (t d) -> p t d", d=64),
                        op=OP.add)
                    rb = bass.AP(recip[:].tensor, recip[:].offset, [[4, 128], [1, 4], [0, 64]])
                    nc.vector.tensor_tensor(out=osl, in0=osl, in1=rb, op=OP.mult)
                    # store: out_s[a, (4s+g)*64+d] -> o[128*(4s+g)+a, d]
                    dstap = bass.AP(o_d.ap().tensor, 64 * 128 * t0,
                                    [[64, 128], [128 * 64, 4], [1, 64]])
                    nc.sync.dma_start(out=dstap, in_=out_s[:, 64 * t0:64 * t1])

                for u in range(NT + 3):
                    if u < NT:
                        stageA(u)
                    if 2 <= u < NT + 2:
                        stageB(u - 2)
                    if u >= 3:
                        stageC(u - 3)
                live["stageO"] = stageO

              with (
                  tc.tile_pool(name="o_ps", bufs=2, space="PSUM") as op_,
                  tc.tile_pool(name="ob_ps", bufs=2, space="PSUM") as obp,
              ):
                for s in range(4):
                    live["stageO"](s, op_, obp)

    nc.compile()
    return nc


def _make_runner(nc, n_cores):
    import concourse.mybir as mybir
    import jax
    from jax.sharding import Mesh, PartitionSpec
    from jax.experimental.shard_map import shard_map
    from concourse.bass2jax import install_neuronx_cc_hook, _bass_exec_p, partition_id_tensor

    install_neuronx_cc_hook()
    partition_name = nc.partition_id_tensor.name if nc.partition_id_tensor else None
    in_names, out_names, out_avals, zero_outs = [], [], [], []
    for alloc in nc.m.functions[0].allocations:
        if not isinstance(alloc, mybir.MemoryLocationSet):
            continue
        name = alloc.memorylocations[0].name
        if alloc.kind == "ExternalInput":
            if name != partition_name:
                in_names.append(name)
        elif alloc.kind == "ExternalOutput":
            shape = tuple(alloc.tensor_shape)
            dtype = mybir.dt.np(alloc.dtype)
            out_names.append(name)
            out_avals.append(jax.core.ShapedArray(shape, dtype))
            zero_outs.append(np.zeros(shape, dtype))
    n_params = len(in_names)
    n_outs = len(out_avals)
    all_in_names = list(in_names) + list(out_names)
    if partition_name is not None:
        all_in_names.append(partition_name)

    def _body(*args):
        operands = list(args)
        if partition_name is not None:
            operands.append(partition_id_tensor())
        outs = _bass_exec_p.bind(
            *operands, out_avals=tuple(out_avals), in_names=tuple(all_in_names),
            out_names=tuple(out_names), lowering_input_output_aliases=(),
            sim_require_finite=True, sim_require_nnan=True, nc=nc)
        return tuple(outs)

    devices = jax.devices()[:n_cores]
    mesh = Mesh(np.asarray(devices), ("core",))
    in_specs = (PartitionSpec("core"),) * (n_params + n_outs)
    out_specs = (PartitionSpec("core"),) * n_outs
    jitted = jax.jit(shard_map(_body, mesh=mesh, in_specs=in_specs,
                               out_specs=out_specs, check_rep=False), keep_unused=True)

    def run(in_maps):
        concat_in = [np.concatenate([np.asarray(in_maps[c][n]) for c in range(n_cores)], axis=0)
                     for n in in_names]
        concat_zeros = [np.zeros((n_cores * z.shape[0], *z.shape[1:]), z.dtype) for z in zero_outs]
        outs = jitted(*concat_in, *concat_zeros)
        import jax as _jax
        _jax.block_until_ready(outs)
        return [{name: np.asarray(outs[i]).reshape(n_cores, *out_avals[i].shape)[c]
                 for i, name in enumerate(out_names)} for c in range(n_cores)]
    return run


def _get_runner(causal: bool):
    key = bool(causal)
    if key not in _CACHE:
        nc = _build_program(key)
        _CACHE[key] = _make_runner(nc, H)
    return _CACHE[key]


def _prep_head(q2, k2, v2, rpe, causal):
    """q2,k2,v2: [N, D] fp32 for one head. Returns per-core input dict."""
    qT = np.ascontiguousarray(q2.T)                      # [64, N]
    kT = np.ascontiguousarray(k2.T)
    # v with ones col 64 (denominator) and zero col 65 (f32r even-width pad)
    v3 = np.concatenate([v2, np.ones((N, 1), np.float32),
                         np.zeros((N, 1), np.float32)], axis=1)  # [N, 66]
    v_r = np.ascontiguousarray(
        v3.reshape(NJC, 128, 66).transpose(1, 0, 2).reshape(128, NJC * 66))
    if causal:
        vc = np.cumsum(v2, axis=0, dtype=np.float64).astype(np.float32)
    else:
        vc = np.broadcast_to(v2.sum(axis=0, dtype=np.float64).astype(np.float32), (N, 64))
    vcum = np.ascontiguousarray(
        vc.reshape(NT, 128, 64).transpose(1, 0, 2).reshape(128, NT * 64))
    return {"qT": qT, "kT": kT, "vr": v_r, "vcum": vcum}


def kernel(q, k, v, rpe_matrix, mask):
    causal = bool(np.asarray(mask).item()) if not isinstance(mask, (int, bool)) else bool(mask)
    q = np.asarray(q, dtype=np.float32)
    k = np.asarray(k, dtype=np.float32)
    v = np.asarray(v, dtype=np.float32)
    rpe = np.asarray(rpe_matrix, dtype=np.float32)

    RPW = 2560 if causal else 4608
    if causal:
        # u in [0, N-1]: rpe_rev[u] = rpe[2N-2-u] -> rows 2N-2 .. N-1
        rpe_rev = rpe[N - 1:2 * N - 1][::-1]             # [N, 64]
    else:
        rpe_rev = rpe[::-1]                              # [2N-1, 64]
    rpeT = np.zeros((64, RPW), dtype=np.float32)
    rpeT[:, :rpe_rev.shape[0]] = rpe_rev.T

    a = np.arange(128, dtype=np.float32)[:, None]
    tt = np.arange(NT, dtype=np.float32)[None, :]
    iota = (128 * tt + a + 1.0) if causal else np.full((128, NT), float(N), np.float32)
    iota = np.ascontiguousarray(iota.astype(np.float32))

    run = _get_runner(causal)
    in_maps = []
    for h in range(H):
        m = _prep_head(q[0, h], k[0, h], v[0, h], rpe, causal)
        m["rpeT"] = rpeT
        m["iota"] = iota
        in_maps.append(m)
    results = run(in_maps)
    out = np.stack([results[h]["o"] for h in range(H)])[None]  # [1, H, N, 64]
    return out.astype(np.float32)


if __name__ == "__main__":
    rng = np.random.default_rng(0)
    q = rng.standard_normal((B, H, N, D), dtype=np.float32)
    k = rng.standard_normal((B, H, N, D), dtype=np.float32)
    v = rng.standard_normal((B, H, N, D), dtype=np.float32)
    rpe = rng.standard_normal((2 * N - 1, D), dtype=np.float32)
    o = kernel(q, k, v, rpe, 1)
    print("out", o.shape, o.dtype, np.abs(o).mean())



# revision 55
# speedup vs baseline: 1.1239x; 1.1239x over previous
"""Trainium2 Bass kernel for FASTMultiHeadAttention (fastmax, Taylor-2 softmax approx
with relative positional embeddings, optional causal mask).

B=1, H=8, N=2048, D=64. One head per NeuronCore (8 cores).

Math per head (q,k,v: [N,D], rpe: [2N-1, D]):
    s[i,j]  = q_i.k_j + q_i.rpe[i-j+N-1]
    w       = 1 + s + s^2/2      (causal-masked if mask)
    out_i   = sum_j w[i,j] v_j / sum_j w[i,j]

Device algorithm (per head):
    w = ((s+1)^2 + 1)/2 on valid entries, so with t = (s+1)^2 (t=0 on masked):
      numer_i = 0.5*(sum_j t_ij v_j + vcum_i)
      denom_i = 0.5*(sum_j t_ij + (i+1))
    The +1 inside the square comes from a 65th "ones" contraction row: qT/kT
    carry a ones row (rpe a zeros row), so the content matmul yields q.k + 1
    and the full score s1 = (q.k + 1) + q.rpe needs only a tensor_tensor add.

    - content+rpe scores: K=65 f32r matmuls with bf16 PSUM output
    - rpe diagonal realignment R[a,j] = QR[a, 127-a+j]: skewed SBUF->SBUF DMA
    - s1 = S_psum + R  via bf16 tensor_tensor (DVE 2x mode)
    - causal mask: affine_select zeroes j > i on the diagonal chunk (Pool)
    - W^T via PE transposes (bf16), squared during PSUM evacuation
      (DVE tensor_tensor self-mult at 2x, or ACT Square)
    - O = sum_j t v via PE matmul with V (+ones col) stationary, K=128
    - normalize with host-precomputed vcum/iota, store [128, NT*64] row-major
"""

import sys
import os
import numpy as np

for _p in ("/opt/trn_rl_repo", "/root/.axon_site/_ro/trn_rl_repo"):
    if os.path.isdir(_p) and _p not in sys.path:
        sys.path.insert(0, _p)

B, H, N, D = 1, 8, 2048, 64
NT = N // 128            # 16 i-tiles of 128 rows
NJC = N // 128           # 16 j-chunks of 128 (for transposes / O matmul)

_CACHE = {}

# engine-assignment tuning (fractions routed to the listed engine)
TUNE = {
    "sq_act_frac": 1.0,     # (unused; squares are ACT-only, PSUM 1-input rule)
    "qr_dve_frac": 0.5,     # QR psum->sbuf copies on DVE (else ACT)
    "qr_pool_frac": 0.0,    # unused: GPSIMD cannot access PSUM
    "s1_pool_frac": 0.0,    # s1 TT chunks on Pool (else DVE)
    "gather_act_frac": 0.0, # gathers issued from ACT ring (else SP)
    "norm_pool": True,      # normalize adds on Pool (else DVE)
}


class _Frac:
    # weighted deterministic router: pick() True with rate `frac`
    def __init__(self, frac):
        self.f = frac
        self.acc = 0.0

    def pick(self):
        self.acc += self.f
        if self.acc >= 0.999:
            self.acc -= 1.0
            return True
        return False


def _build_program(causal: bool, reps: int = 1):
    import concourse.bass as bass
    from concourse import bacc
    import concourse.mybir as mybir
    from concourse.tile import TileContext
    from concourse.masks import make_identity

    fp32 = mybir.dt.float32
    f32r = mybir.dt.float32r
    bf16 = mybir.dt.bfloat16
    AT = mybir.ActivationFunctionType
    OP = mybir.AluOpType

    RPW = 2560 if causal else 4608   # rpe_revT padded width

    nc = bacc.Bacc("TRN2", target_bir_lowering=False, debug=False)

    qT_d = nc.dram_tensor("qT", [65, N], f32r, kind="ExternalInput")
    kT_d = nc.dram_tensor("kT", [65, N], f32r, kind="ExternalInput")
    v_d = nc.dram_tensor("vr", [128, NJC * 66], bf16, kind="ExternalInput")
    vcum_d = nc.dram_tensor("vcum", [128, NT * 64], fp32, kind="ExternalInput")
    rpe_d = nc.dram_tensor("rpeT", [65, RPW], f32r, kind="ExternalInput")
    iota_d = nc.dram_tensor("iota", [128, NT], fp32, kind="ExternalInput")
    o_d = nc.dram_tensor("o", [128, NT * 64], fp32, kind="ExternalOutput")

    def j_max(t):
        return 128 * (t + 1) if causal else N

    def u_min(t):
        return (N - 1) - 128 * t - 127

    def qr_w(t):
        return 127 + j_max(t)

    with TileContext(nc) as tc:
        with (
            tc.tile_pool(name="persist", bufs=1) as pp,
            tc.tile_pool(name="qr", bufs=3) as qrp,
            tc.tile_pool(name="rr", bufs=6) as rrp,
            tc.tile_pool(name="s1", bufs=4) as s1p,
            tc.tile_pool(name="small", bufs=2) as sp,
        ):
            sq_r = _Frac(TUNE["sq_act_frac"])
            qrd_r = _Frac(TUNE["qr_dve_frac"])
            qrp_r = _Frac(TUNE["qr_pool_frac"])
            s1p_r = _Frac(TUNE["s1_pool_frac"])
            ga_r = _Frac(TUNE["gather_act_frac"])

            # ---- persistent tiles ----
            qT_s = pp.tile([65, N], f32r, name="qT_s")
            kT_s = pp.tile([65, N], f32r, name="kT_s")
            rpe_s = pp.tile([65, RPW], f32r, name="rpe_s")
            v_s = pp.tile([128, NJC * 66], bf16, name="v_s")
            vcum_s = pp.tile([128, NT * 64], fp32, name="vcum_s")
            iota_s = pp.tile([128, NT], fp32, name="iota_s")

            # chunked loads, ordered by pipeline consumption under the
            # interleaved tile order (small tile t, then tile t+8, ...)
            if causal:
                rpe_chunks = ((1920, RPW), (896, 1920), (0, 896))
            else:
                rpe_chunks = ((896, RPW), (0, 896))
            qT_chunks = ((0, 256), (1024, 1280), (256, 1024), (1280, 2048))
            kT_chunks = ((0, 256), (256, 1280), (1280, 2048))
            # fill-critical chunks on SP first (tiles 0 and 8 consume them
            # within the first two iterations); the rest on Pool SWDGE / ACT
            nc.sync.dma_start(out=qT_s[:, 0:256], in_=qT_d.ap()[:, 0:256])
            nc.sync.dma_start(out=rpe_s[:, rpe_chunks[0][0]:rpe_chunks[0][1]],
                              in_=rpe_d.ap()[:, rpe_chunks[0][0]:rpe_chunks[0][1]])
            nc.sync.dma_start(out=qT_s[:, 1024:1280], in_=qT_d.ap()[:, 1024:1280])
            nc.sync.dma_start(out=kT_s[:, 0:256], in_=kT_d.ap()[:, 0:256])
            for c0, c1 in rpe_chunks[1:]:
                nc.sync.dma_start(out=rpe_s[:, c0:c1], in_=rpe_d.ap()[:, c0:c1])
            for c0, c1 in kT_chunks[1:]:
                nc.sync.dma_start(out=kT_s[:, c0:c1], in_=kT_d.ap()[:, c0:c1])
            nc.scalar.dma_start(out=v_s[:], in_=v_d.ap())
            for c0, c1 in qT_chunks[2:]:
                nc.sync.dma_start(out=qT_s[:, c0:c1], in_=qT_d.ap()[:, c0:c1])
            nc.scalar.dma_start(out=vcum_s[:], in_=vcum_d.ap())
            nc.scalar.dma_start(out=iota_s[:], in_=iota_d.ap())

            ident = pp.tile([128, 128], bf16, name="ident")
            make_identity(nc, ident[:])
            ident66_f = pp.tile([66, 66], fp32, name="ident66_f")
            make_identity(nc, ident66_f[:])
            ident66_r = pp.tile([66, 66], f32r, name="ident66_r")
            nc.vector.tensor_copy(ident66_r[:], ident66_f[:])
            ident66 = ident66_r[:]

            # W^T storage, triangular-packed by groups of 4 j-chunks when causal:
            # group g0 stores only i >= 128*g0 (width Wg = N - 128*g0).
            def wt_imin(jc):
                return 128 * (4 * (jc // 4)) if causal else 0

            def wt_w(jc):
                return N - wt_imin(jc)

            _wt_base = {}
            _off = 0
            for _jc in range(NJC):
                _wt_base[_jc] = _off
                _off += wt_w(_jc)
            WTW = _off
            wt_all = pp.tile([128, WTW], bf16, name="wt_all")

            # accumulator [128, NT*65]: per tile 64 numerator cols + denom col
            O_all = pp.tile([128, NT * 65], fp32, name="O_all")
            out_s = pp.tile([128, NT * 64], fp32, name="out_s")

            for _rep in range(reps):
              with (
                  tc.tile_pool(name="qr_ps", bufs=2, space="PSUM") as qrps,
                  tc.tile_pool(name="s_ps", bufs=2, space="PSUM") as sps,
                  tc.tile_pool(name="tr_ps", bufs=2, space="PSUM") as trp,
              ):
                live = {}

                def mm65(out_ps, t, src, c0, mw):
                    i0 = 128 * t
                    nc.tensor.matmul(out_ps, qT_s[:, i0:i0 + 128],
                                     src[:, c0:c0 + mw],
                                     start=True, stop=True, tile_position=(0, 0))

                def stageA(t):
                    # rpe projection QR (K=65, zero row kills the ones term),
                    # fp32 psum, ACT evac to bf16, then diagonal gather of R
                    w = qr_w(t)
                    um = u_min(t)
                    qrbuf = qrp.tile([128, 2560 if causal else 2304], bf16, name="qrbuf")
                    for b0 in range(0, w, 1024):
                        bw = min(1024, w - b0)
                        qr_ps = qrps.tile([128, 1024], fp32, name="qr_ps")
                        for h0 in range(0, bw, 512):
                            hw = min(512, bw - h0)
                            mw = max(256, (hw + 1) & ~1)  # f32r ISA: even, >= 256
                            mm65(qr_ps[:, h0:h0 + mw], t, rpe_s, um + b0 + h0, mw)
                        if qrd_r.pick():
                            nc.vector.tensor_copy(qrbuf[:, b0:b0 + bw], qr_ps[:, 0:bw])
                        elif qrp_r.pick():
                            nc.gpsimd.tensor_copy(qrbuf[:, b0:b0 + bw], qr_ps[:, 0:bw])
                        else:
                            nc.scalar.activation(qrbuf[:, b0:b0 + bw], qr_ps[:, 0:bw],
                                                 AT.Copy, bias=0.0, scale=1.0)
                    # diagonal gather R[a, j] = qrbuf[a, 127 - a + j]
                    QW = qrbuf[:].tensor.shape[1]
                    R_row = rrp.tile([128, N], bf16, name="R_row")
                    diag = bass.AP(qrbuf[:].tensor, qrbuf[:].offset + 127,
                                   [[QW - 1, 128], [1, j_max(t)]])
                    eng = nc.scalar if ga_r.pick() else nc.sync
                    eng.dma_start(out=R_row[:, 0:j_max(t)], in_=diag)
                    live[("A", t)] = R_row

                def stageB(t):
                    # content scores (K=65 with ones row -> q.k + 1), bf16 psum,
                    # s1 = S + R via DVE tensor_tensor (2x), causal mask on diag
                    i0 = 128 * t
                    jm = j_max(t)
                    R_row = live.pop(("A", t))
                    s1_row = s1p.tile([128, N], bf16, name="s1_row", tag="s1_row")
                    for jb in range(0, jm, 512):
                        cw = min(512, jm - jb)
                        s_ps = sps.tile([128, 512], fp32, name="s_ps")
                        mw = max(256, (cw + 1) & ~1)
                        mm65(s_ps[:, 0:mw], t, kT_s, jb, mw)
                        teng = nc.gpsimd if s1p_r.pick() else nc.vector
                        teng.tensor_tensor(
                            out=s1_row[:, jb:jb + cw], in0=s_ps[:, 0:cw],
                            in1=R_row[:, jb:jb + cw], op=OP.add)
                    s1_diag = None
                    if causal:
                        # masked diagonal chunk goes to its own tile so the mask
                        # doesn't gate the other chunks' transposes
                        s1_diag = s1p.tile([128, 128], bf16, name="s1_diag", tag="s1_diag")
                        nc.gpsimd.affine_select(
                            out=s1_diag[:], in_=s1_row[:, i0:i0 + 128],
                            compare_op=OP.is_ge, fill=0.0,
                            base=0, channel_multiplier=1, pattern=[[-1, 128]])
                    live[("B", t)] = (s1_diag, s1_row)

                def stageC(t):
                    # transpose s1 chunks, square during PSUM evacuation -> wt_all
                    i0 = 128 * t
                    s1_diag, s1_row = live.pop(("B", t))
                    njc = (j_max(t) + 127) // 128
                    for g0 in range(0, njc, 4):
                        gn = min(4, njc - g0)
                        tr_ps = trp.tile([128, 512], bf16, name="tr_ps")
                        for g in range(gn):
                            jc = g0 + g
                            src_chunk = (s1_diag[:] if (causal and jc == t)
                                         else s1_row[:, 128 * jc:128 * (jc + 1)])
                            nc.tensor.transpose(tr_ps[:, 128 * g:128 * (g + 1)],
                                                src_chunk, ident[:])
                        dst = bass.AP(wt_all[:].tensor,
                                      wt_all[:].offset + _wt_base[g0] + (i0 - wt_imin(g0)),
                                      [[WTW, 128], [wt_w(g0), gn], [1, 128]])
                        srcap = tr_ps[:, 0:128 * gn].rearrange("p (g c) -> p g c", g=gn)
                        # PSUM allows only one tensor input per instruction, so
                        # the square must be ACT's single-input Square
                        nc.scalar.activation(dst, srcap, AT.Square, bias=0.0, scale=1.0)

                def stageOacc(t):
                    # accumulate O for i-range [128t, 128t+128) over its j-chunks
                    # right after stageC(t) wrote those W^T columns; transient
                    # psum partial, evacuated straight to the slab SBUF tile
                    s = t // 4
                    if ("O", s) not in live:
                        live[("O", s)] = sp.tile([66, 512], f32r, name="oT_s", tag="oT_s")
                    oT_s = live[("O", s)]
                    c0 = 128 * (t % 4)
                    o_ps = sps.tile([66, 128], fp32, name="s_ps")
                    jc_hi = t + 1 if causal else NJC
                    for jc in range(jc_hi):
                        rhs = bass.AP(wt_all[:].tensor,
                                      wt_all[:].offset + _wt_base[jc] + (128 * t - wt_imin(jc)),
                                      [[WTW, 128], [1, 128]])
                        nc.tensor.matmul(o_ps[:, 0:128],
                                         v_s[:, 66 * jc:66 * (jc + 1)], rhs,
                                         start=(jc == 0), stop=(jc == jc_hi - 1))
                    nc.scalar.activation(oT_s[:, c0:c0 + 128], o_ps[:, 0:128],
                                         AT.Copy, bias=0.0, scale=1.0)

                def stageOfin(s):
                    # back-transpose + normalize + store slab s
                    t0, t1 = 4 * s, 4 * s + 4
                    oT_s = live.pop(("O", s))
                    ob_ps = qrps.tile([128, 264], f32r, name="qr_ps")
                    for g in range(4):
                        nc.tensor.transpose(ob_ps[:, 66 * g:66 * (g + 1)],
                                            oT_s[:, 128 * g:128 * (g + 1)],
                                            ident66)
                    odst = bass.AP(O_all[:].tensor, O_all[:].offset + 65 * t0,
                                   [[NT * 65, 128], [65, 4], [1, 65]])
                    osrc = ob_ps[:, 0:264].rearrange("p (g c) -> p g c", g=4)[:, :, 0:65]
                    nc.vector.tensor_copy(odst, osrc)
                    # normalize this slab: denom col 64 + iota -> recip; out = (num+vcum)*recip
                    dtot = sp.tile([128, 4], fp32, name="dtot", tag="dtot")
                    dcol = bass.AP(O_all[:].tensor, O_all[:].offset + 65 * t0 + 64,
                                   [[NT * 65, 128], [65, 4]])
                    nc.vector.tensor_tensor(out=dtot[:], in0=dcol, in1=iota_s[:, t0:t1], op=OP.add)
                    recip = sp.tile([128, 4], fp32, name="recip", tag="recip")
                    nc.vector.reciprocal(recip[:], dtot[:])
                    onum = bass.AP(O_all[:].tensor, O_all[:].offset + 65 * t0,
                                   [[NT * 65, 128], [65, 4], [1, 64]])
                    osl = out_s[:, 64 * t0:64 * t1].rearrange("p (t d) -> p t d", d=64)
                    neng = nc.gpsimd if TUNE["norm_pool"] else nc.vector
                    neng.tensor_tensor(
                        out=osl, in0=onum,
                        in1=vcum_s[:, 64 * t0:64 * t1].rearrange("p (t d) -> p t d", d=64),
                        op=OP.add)
                    rb = bass.AP(recip[:].tensor, recip[:].offset, [[4, 128], [1, 4], [0, 64]])
                    neng.tensor_tensor(out=osl, in0=osl, in1=rb, op=OP.mult)
                    nc.gpsimd.dma_start(out=o_d.ap()[:, 64 * t0:64 * t1],
                                        in_=out_s[:, 64 * t0:64 * t1])

                # interleaved tile order pairs small and large tiles so the
                # per-iteration engine load is roughly uniform
                order = [t for pair in zip(range(NT // 2), range(NT // 2, NT))
                         for t in pair]
                slab_done = {s: 0 for s in range(NT // 4)}
                for u in range(NT + 4):
                    if 2 <= u < NT + 2:
                        stageB(order[u - 2])
                    if u < NT:
                        stageA(order[u])
                    if u >= 4:
                        t = order[u - 4]
                        stageC(t)
                        stageOacc(t)
                        slab_done[t // 4] += 1
                        if slab_done[t // 4] == 4:
                            stageOfin(t // 4)

    nc.compile()
    return nc


def _make_runner(nc, n_cores):
    import concourse.mybir as mybir
    import jax
    from jax.sharding import Mesh, PartitionSpec
    from jax.experimental.shard_map import shard_map
    from concourse.bass2jax import install_neuronx_cc_hook, _bass_exec_p, partition_id_tensor

    install_neuronx_cc_hook()
    partition_name = nc.partition_id_tensor.name if nc.partition_id_tensor else None
    in_names, out_names, out_avals, zero_outs = [], [], [], []
    for alloc in nc.m.functions[0].allocations:
        if not isinstance(alloc, mybir.MemoryLocationSet):
            continue
        name = alloc.memorylocations[0].name
        if alloc.kind == "ExternalInput":
            if name != partition_name:
                in_names.append(name)
        elif alloc.kind == "ExternalOutput":
            shape = tuple(alloc.tensor_shape)
            dtype = mybir.dt.np(alloc.dtype)
            out_names.append(name)
            out_avals.append(jax.core.ShapedArray(shape, dtype))
            zero_outs.append(np.zeros(shape, dtype))
    n_params = len(in_names)
    n_outs = len(out_avals)
    all_in_names = list(in_names) + list(out_names)
    if partition_name is not None:
        all_in_names.append(partition_name)

    def _body(*args):
        operands = list(args)
        if partition_name is not None:
            operands.append(partition_id_tensor())
        outs = _bass_exec_p.bind(
            *operands, out_avals=tuple(out_avals), in_names=tuple(all_in_names),
            out_names=tuple(out_names), lowering_input_output_aliases=(),
            sim_require_finite=True, sim_require_nnan=True, nc=nc)
        return tuple(outs)

    devices = jax.devices()[:n_cores]
    mesh = Mesh(np.asarray(devices), ("core",))
    in_specs = (PartitionSpec("core"),) * (n_params + n_outs)
    out_specs = (PartitionSpec("core"),) * n_outs
    jitted = jax.jit(shard_map(_body, mesh=mesh, in_specs=in_specs,
                               out_specs=out_specs, check_rep=False), keep_unused=True)

    def run(in_maps):
        concat_in = [np.concatenate([np.asarray(in_maps[c][n]) for c in range(n_cores)], axis=0)
                     for n in in_names]
        concat_zeros = [np.zeros((n_cores * z.shape[0], *z.shape[1:]), z.dtype) for z in zero_outs]
        outs = jitted(*concat_in, *concat_zeros)
        import jax as _jax
        _jax.block_until_ready(outs)
        return [{name: np.asarray(outs[i]).reshape(n_cores, *out_avals[i].shape)[c]
                 for i, name in enumerate(out_names)} for c in range(n_cores)]
    return run


def _get_runner(causal: bool):
    key = bool(causal)
    if key not in _CACHE:
        nc = _build_program(key)
        _CACHE[key] = _make_runner(nc, H)
    return _CACHE[key]


def _prep_head(q2, k2, v2, causal):
    """q2,k2,v2: [N, D] fp32 for one head. Returns per-core input dict."""
    import ml_dtypes
    qT = np.concatenate([q2.T, np.ones((1, N), np.float32)], axis=0)  # [65, N]
    kT = np.concatenate([k2.T, np.ones((1, N), np.float32)], axis=0)
    # v with ones col 64 (denominator) and zero col 65 (f32r even-width pad)
    v3 = np.concatenate([v2, np.ones((N, 1), np.float32),
                         np.zeros((N, 1), np.float32)], axis=1)      # [N, 66]
    v_r = np.ascontiguousarray(
        v3.reshape(NJC, 128, 66).transpose(1, 0, 2).reshape(128, NJC * 66)
    ).astype(ml_dtypes.bfloat16)
    if causal:
        vc = np.cumsum(v2, axis=0, dtype=np.float64).astype(np.float32)
    else:
        vc = np.broadcast_to(v2.sum(axis=0, dtype=np.float64).astype(np.float32), (N, 64))
    vcum = np.ascontiguousarray(
        vc.reshape(NT, 128, 64).transpose(1, 0, 2).reshape(128, NT * 64))
    return {"qT": np.ascontiguousarray(qT), "kT": np.ascontiguousarray(kT),
            "vr": v_r, "vcum": vcum}


def kernel(q, k, v, rpe_matrix, mask):
    causal = bool(np.asarray(mask).item()) if not isinstance(mask, (int, bool)) else bool(mask)
    q = np.asarray(q, dtype=np.float32)
    k = np.asarray(k, dtype=np.float32)
    v = np.asarray(v, dtype=np.float32)
    rpe = np.asarray(rpe_matrix, dtype=np.float32)

    RPW = 2560 if causal else 4608
    if causal:
        # u in [0, N-1]: rpe_rev[u] = rpe[2N-2-u] -> rows 2N-2 .. N-1
        rpe_rev = rpe[N - 1:2 * N - 1][::-1]             # [N, 64]
    else:
        rpe_rev = rpe[::-1]                              # [2N-1, 64]
    rpeT = np.zeros((65, RPW), dtype=np.float32)
    rpeT[0:64, :rpe_rev.shape[0]] = rpe_rev.T            # row 64 stays zero

    a = np.arange(128, dtype=np.float32)[:, None]
    tt = np.arange(NT, dtype=np.float32)[None, :]
    iota = (128 * tt + a + 1.0) if causal else np.full((128, NT), float(N), np.float32)
    iota = np.ascontiguousarray(iota.astype(np.float32))

    run = _get_runner(causal)
    in_maps = []
    for h in range(H):
        m = _prep_head(q[0, h], k[0, h], v[0, h], causal)
        m["rpeT"] = rpeT
        m["iota"] = iota
        in_maps.append(m)
    results = run(in_maps)
    # o stored [128, NT*64] with o_store[a, 64t+d] = o[128t+a, d]
    outs = []
    for h in range(H):
        oh = results[h]["o"].reshape(128, NT, 64).transpose(1, 0, 2).reshape(N, 64)
        outs.append(oh)
    out = np.stack(outs)[None]  # [1, H, N, 64]
    return out.astype(np.float32)


if __name__ == "__main__":
    rng = np.random.default_rng(0)
    q = rng.standard_normal((B, H, N, D), dtype=np.float32)
    k = rng.standard_normal((B, H, N, D), dtype=np.float32)
    v = rng.standard_normal((B, H, N, D), dtype=np.float32)
    rpe = rng.standard_normal((2 * N - 1, D), dtype=np.float32)
    o = kernel(q, k, v, rpe, 1)
    print("out", o.shape, o.dtype, np.abs(o).mean())


# revision 61
# speedup vs baseline: 1.1600x; 1.0321x over previous
"""Trainium2 Bass kernel for FASTMultiHeadAttention (fastmax, Taylor-2 softmax approx
with relative positional embeddings, optional causal mask).

B=1, H=8, N=2048, D=64. One head per NeuronCore (8 cores).

Math per head (q,k,v: [N,D], rpe: [2N-1, D]):
    s[i,j]  = q_i.k_j + q_i.rpe[i-j+N-1]
    w       = 1 + s + s^2/2      (causal-masked if mask)
    out_i   = sum_j w[i,j] v_j / sum_j w[i,j]

Device algorithm (per head):
    w = ((s+1)^2 + 1)/2 on valid entries, so with t = (s+1)^2 (t=0 on masked):
      numer_i = 0.5*(sum_j t_ij v_j + vcum_i)
      denom_i = 0.5*(sum_j t_ij + (i+1))
    The +1 inside the square comes from a 65th "ones" contraction row: qT/kT
    carry a ones row (rpe a zeros row), so the content matmul yields q.k + 1
    and the full score s1 = (q.k + 1) + q.rpe needs only a tensor_tensor add.

    - content+rpe scores: K=65 f32r matmuls with bf16 PSUM output
    - rpe diagonal realignment R[a,j] = QR[a, 127-a+j]: skewed SBUF->SBUF DMA
    - s1 = S_psum + R  via bf16 tensor_tensor (DVE 2x mode)
    - causal mask: affine_select zeroes j > i on the diagonal chunk (Pool)
    - W^T via PE transposes (bf16), squared during PSUM evacuation
      (DVE tensor_tensor self-mult at 2x, or ACT Square)
    - O = sum_j t v via PE matmul with V (+ones col) stationary, K=128
    - normalize with host-precomputed vcum/iota, store [128, NT*64] row-major
"""

import sys
import os
import numpy as np

for _p in ("/opt/trn_rl_repo", "/root/.axon_site/_ro/trn_rl_repo"):
    if os.path.isdir(_p) and _p not in sys.path:
        sys.path.insert(0, _p)

B, H, N, D = 1, 8, 2048, 64
NT = N // 128            # 16 i-tiles of 128 rows
NJC = N // 128           # 16 j-chunks of 128 (for transposes / O matmul)

_CACHE = {}

# engine-assignment tuning (fractions routed to the listed engine)
TUNE = {
    "sq_act_frac": 1.0,     # (unused; squares are ACT-only, PSUM 1-input rule)
    "qr_dve_frac": 0.48,     # QR psum->sbuf copies on DVE (else ACT)
    "qr_pool_frac": 0.0,    # unused: GPSIMD cannot access PSUM
    "s1_pool_frac": 0.0,    # s1 TT chunks on Pool (else DVE)
    "gather_act_frac": 0.0, # gathers issued from ACT ring (else SP)
    "norm_pool": True,      # normalize adds on Pool (else DVE)
}


class _Frac:
    # weighted deterministic router: pick() True with rate `frac`
    def __init__(self, frac):
        self.f = frac
        self.acc = 0.0

    def pick(self):
        self.acc += self.f
        if self.acc >= 0.999:
            self.acc -= 1.0
            return True
        return False


def _build_program(causal: bool, reps: int = 1):
    import concourse.bass as bass
    from concourse import bacc
    import concourse.mybir as mybir
    from concourse.tile import TileContext
    from concourse.masks import make_identity

    fp32 = mybir.dt.float32
    f32r = mybir.dt.float32r
    bf16 = mybir.dt.bfloat16
    AT = mybir.ActivationFunctionType
    OP = mybir.AluOpType

    RPW = 2560 if causal else 4608   # rpe_revT padded width

    nc = bacc.Bacc("TRN2", target_bir_lowering=False, debug=False)

    qT_d = nc.dram_tensor("qT", [65, N], f32r, kind="ExternalInput")
    kT_d = nc.dram_tensor("kT", [65, N], f32r, kind="ExternalInput")
    v_d = nc.dram_tensor("vr", [128, NJC * 66], bf16, kind="ExternalInput")
    vcum_d = nc.dram_tensor("vcum", [128, NT * 64], fp32, kind="ExternalInput")
    rpe_d = nc.dram_tensor("rpeT", [65, RPW], f32r, kind="ExternalInput")
    iota_d = nc.dram_tensor("iota", [128, NT], fp32, kind="ExternalInput")
    o_d = nc.dram_tensor("o", [128, NT * 64], fp32, kind="ExternalOutput")

    def j_max(t):
        return 128 * (t + 1) if causal else N

    def u_min(t):
        return (N - 1) - 128 * t - 127

    def qr_w(t):
        return 127 + j_max(t)

    with TileContext(nc) as tc:
        with (
            tc.tile_pool(name="persist", bufs=1) as pp,
            tc.tile_pool(name="qr", bufs=3) as qrp,
            tc.tile_pool(name="rr", bufs=6) as rrp,
            tc.tile_pool(name="s1", bufs=4) as s1p,
            tc.tile_pool(name="small", bufs=2) as sp,
        ):
            sq_r = _Frac(TUNE["sq_act_frac"])
            qrd_r = _Frac(TUNE["qr_dve_frac"])
            qrp_r = _Frac(TUNE["qr_pool_frac"])
            s1p_r = _Frac(TUNE["s1_pool_frac"])
            ga_r = _Frac(TUNE["gather_act_frac"])

            # ---- persistent tiles ----
            qT_s = pp.tile([65, N], f32r, name="qT_s")
            kT_s = pp.tile([65, N], f32r, name="kT_s")
            rpe_s = pp.tile([65, RPW], f32r, name="rpe_s")
            v_s = pp.tile([128, NJC * 66], bf16, name="v_s")
            vcum_s = pp.tile([128, NT * 64], fp32, name="vcum_s")
            iota_s = pp.tile([128, NT], fp32, name="iota_s")

            # chunked loads, ordered by pipeline consumption under the
            # interleaved tile order (small tile t, then tile t+8, ...)
            if causal:
                rpe_chunks = ((1920, 2176), (896, 1920), (0, 896), (2176, 2304))
            else:
                rpe_chunks = ((896, RPW), (0, 896))
            qT_chunks = ((0, 128), (1024, 1152), (128, 1024), (1152, 2048))
            kT_chunks = ((0, 256), (256, 1280), (1280, 2048))
            # fill-critical chunks on SP first (tiles 0 and 8 consume them
            # within the first two iterations); the rest on Pool SWDGE / ACT
            nc.sync.dma_start(out=qT_s[:, 0:128], in_=qT_d.ap()[:, 0:128])
            nc.scalar.dma_start(out=rpe_s[:, rpe_chunks[0][0]:rpe_chunks[0][1]],
                                in_=rpe_d.ap()[:, rpe_chunks[0][0]:rpe_chunks[0][1]])
            nc.sync.dma_start(out=qT_s[:, 1024:1152], in_=qT_d.ap()[:, 1024:1152])
            nc.sync.dma_start(out=kT_s[:, 0:256], in_=kT_d.ap()[:, 0:256])
            nc.sync.dma_start(out=rpe_s[:, rpe_chunks[1][0]:rpe_chunks[1][1]],
                              in_=rpe_d.ap()[:, rpe_chunks[1][0]:rpe_chunks[1][1]])
            for c0, c1 in kT_chunks[1:]:
                nc.sync.dma_start(out=kT_s[:, c0:c1], in_=kT_d.ap()[:, c0:c1])
            nc.gpsimd.dma_start(out=v_s[:], in_=v_d.ap())
            for c0, c1 in qT_chunks[2:]:
                nc.sync.dma_start(out=qT_s[:, c0:c1], in_=qT_d.ap()[:, c0:c1])
            for c0, c1 in rpe_chunks[2:]:
                nc.sync.dma_start(out=rpe_s[:, c0:c1], in_=rpe_d.ap()[:, c0:c1])
            nc.gpsimd.dma_start(out=vcum_s[:], in_=vcum_d.ap())
            nc.gpsimd.dma_start(out=iota_s[:], in_=iota_d.ap())

            ident = pp.tile([128, 128], bf16, name="ident")
            make_identity(nc, ident[:])
            ident66_f = pp.tile([66, 66], fp32, name="ident66_f")
            make_identity(nc, ident66_f[:])
            ident66_r = pp.tile([66, 66], f32r, name="ident66_r")
            nc.vector.tensor_copy(ident66_r[:], ident66_f[:])
            ident66 = ident66_r[:]

            # W^T storage, triangular-packed by groups of 4 j-chunks when causal:
            # group g0 stores only i >= 128*g0 (width Wg = N - 128*g0).
            def wt_imin(jc):
                return 128 * (4 * (jc // 4)) if causal else 0

            def wt_w(jc):
                return N - wt_imin(jc)

            _wt_base = {}
            _off = 0
            for _jc in range(NJC):
                _wt_base[_jc] = _off
                _off += wt_w(_jc)
            WTW = _off
            wt_all = pp.tile([128, WTW], bf16, name="wt_all")

            out_s = pp.tile([128, NT * 64], fp32, name="out_s")

            for _rep in range(reps):
              with (
                  tc.tile_pool(name="qr_ps", bufs=2, space="PSUM") as qrps,
                  tc.tile_pool(name="s_ps", bufs=2, space="PSUM") as sps,
                  tc.tile_pool(name="tr_ps", bufs=2, space="PSUM") as trp,
              ):
                live = {}

                def mm65(out_ps, t, src, c0, mw):
                    i0 = 128 * t
                    nc.tensor.matmul(out_ps, qT_s[:, i0:i0 + 128],
                                     src[:, c0:c0 + mw],
                                     start=True, stop=True, tile_position=(0, 0))

                def stageA(t):
                    # rpe projection QR (K=65, zero row kills the ones term),
                    # fp32 psum, ACT evac to bf16, then diagonal gather of R
                    w = qr_w(t)
                    um = u_min(t)
                    qrbuf = qrp.tile([128, 2560 if causal else 2304], bf16, name="qrbuf")
                    for b0 in range(0, w, 1024):
                        bw = min(1024, w - b0)
                        qr_ps = qrps.tile([128, 1024], fp32, name="qr_ps")
                        for h0 in range(0, bw, 512):
                            hw = min(512, bw - h0)
                            mw = max(256, (hw + 1) & ~1)  # f32r ISA: even, >= 256
                            mm65(qr_ps[:, h0:h0 + mw], t, rpe_s, um + b0 + h0, mw)
                        if qrd_r.pick():
                            nc.vector.tensor_copy(qrbuf[:, b0:b0 + bw], qr_ps[:, 0:bw])
                        elif qrp_r.pick():
                            nc.gpsimd.tensor_copy(qrbuf[:, b0:b0 + bw], qr_ps[:, 0:bw])
                        else:
                            nc.scalar.activation(qrbuf[:, b0:b0 + bw], qr_ps[:, 0:bw],
                                                 AT.Copy, bias=0.0, scale=1.0)
                    # diagonal gather R[a, j] = qrbuf[a, 127 - a + j]
                    QW = qrbuf[:].tensor.shape[1]
                    R_row = rrp.tile([128, N], bf16, name="R_row")
                    diag = bass.AP(qrbuf[:].tensor, qrbuf[:].offset + 127,
                                   [[QW - 1, 128], [1, j_max(t)]])
                    eng = nc.scalar if ga_r.pick() else nc.sync
                    eng.dma_start(out=R_row[:, 0:j_max(t)], in_=diag)
                    live[("A", t)] = R_row

                def stageB(t):
                    # content scores (K=65 with ones row -> q.k + 1), bf16 psum,
                    # s1 = S + R via DVE tensor_tensor (2x), causal mask on diag
                    i0 = 128 * t
                    jm = j_max(t)
                    R_row = live.pop(("A", t))
                    s1_row = s1p.tile([128, N], bf16, name="s1_row", tag="s1_row")
                    for jb in range(0, jm, 512):
                        cw = min(512, jm - jb)
                        s_ps = sps.tile([128, 512], fp32, name="s_ps")
                        mw = max(256, (cw + 1) & ~1)
                        mm65(s_ps[:, 0:mw], t, kT_s, jb, mw)
                        teng = nc.gpsimd if s1p_r.pick() else nc.vector
                        teng.tensor_tensor(
                            out=s1_row[:, jb:jb + cw], in0=s_ps[:, 0:cw],
                            in1=R_row[:, jb:jb + cw], op=OP.add)
                    s1_diag = None
                    if causal:
                        # masked diagonal chunk goes to its own tile so the mask
                        # doesn't gate the other chunks' transposes
                        s1_diag = s1p.tile([128, 128], bf16, name="s1_diag", tag="s1_diag")
                        nc.gpsimd.affine_select(
                            out=s1_diag[:], in_=s1_row[:, i0:i0 + 128],
                            compare_op=OP.is_ge, fill=0.0,
                            base=0, channel_multiplier=1, pattern=[[-1, 128]])
                    live[("B", t)] = (s1_diag, s1_row)

                def stageC(t):
                    # transpose s1 chunks, square during PSUM evacuation -> wt_all
                    i0 = 128 * t
                    s1_diag, s1_row = live.pop(("B", t))
                    njc = (j_max(t) + 127) // 128
                    for g0 in range(0, njc, 4):
                        gn = min(4, njc - g0)
                        tr_ps = trp.tile([128, 512], bf16, name="tr_ps")
                        for g in range(gn):
                            jc = g0 + g
                            src_chunk = (s1_diag[:] if (causal and jc == t)
                                         else s1_row[:, 128 * jc:128 * (jc + 1)])
                            nc.tensor.transpose(tr_ps[:, 128 * g:128 * (g + 1)],
                                                src_chunk, ident[:])
                        dst = bass.AP(wt_all[:].tensor,
                                      wt_all[:].offset + _wt_base[g0] + (i0 - wt_imin(g0)),
                                      [[WTW, 128], [wt_w(g0), gn], [1, 128]])
                        srcap = tr_ps[:, 0:128 * gn].rearrange("p (g c) -> p g c", g=gn)
                        # PSUM allows only one tensor input per instruction, so
                        # the square must be ACT's single-input Square
                        nc.scalar.activation(dst, srcap, AT.Square, bias=0.0, scale=1.0)

                def stageOacc(t):
                    # accumulate O for i-range [128t, 128t+128) over its j-chunks
                    # right after stageC(t) wrote those W^T columns; transient
                    # psum partial, evacuated straight to the slab SBUF tile
                    s = t // 4
                    if ("O", s) not in live:
                        live[("O", s)] = sp.tile([66, 512], f32r, name="oT_s", tag="oT_s")
                    oT_s = live[("O", s)]
                    c0 = 128 * (t % 4)
                    o_ps = sps.tile([66, 128], fp32, name="s_ps")
                    jc_hi = t + 1 if causal else NJC
                    for jc in range(jc_hi):
                        rhs = bass.AP(wt_all[:].tensor,
                                      wt_all[:].offset + _wt_base[jc] + (128 * t - wt_imin(jc)),
                                      [[WTW, 128], [1, 128]])
                        nc.tensor.matmul(o_ps[:, 0:128],
                                         v_s[:, 66 * jc:66 * (jc + 1)], rhs,
                                         start=(jc == 0), stop=(jc == jc_hi - 1))
                    nc.scalar.activation(oT_s[:, c0:c0 + 128], o_ps[:, 0:128],
                                         AT.Copy, bias=0.0, scale=1.0)

                def stageOfin(s):
                    # back-transpose + normalize + store slab s
                    t0, t1 = 4 * s, 4 * s + 4
                    oT_s = live.pop(("O", s))
                    ob_ps = qrps.tile([128, 264], f32r, name="qr_ps")
                    for g in range(4):
                        nc.tensor.transpose(ob_ps[:, 66 * g:66 * (g + 1)],
                                            oT_s[:, 128 * g:128 * (g + 1)],
                                            ident66)
                    # normalize straight from the back-transpose psum (one PSUM
                    # input per instruction is legal on DVE)
                    obf = ob_ps[:].bitcast(fp32)
                    dtot = sp.tile([128, 4], fp32, name="dtot", tag="dtot")
                    dcol = bass.AP(obf.tensor, obf.offset + 64, [[264, 128], [66, 4]])
                    nc.vector.tensor_tensor(out=dtot[:], in0=dcol, in1=iota_s[:, t0:t1], op=OP.add)
                    recip = sp.tile([128, 4], fp32, name="recip", tag="recip")
                    nc.vector.reciprocal(recip[:], dtot[:])
                    onum = bass.AP(obf.tensor, obf.offset, [[264, 128], [66, 4], [1, 64]])
                    osl = out_s[:, 64 * t0:64 * t1].rearrange("p (t d) -> p t d", d=64)
                    nc.vector.tensor_tensor(
                        out=osl, in0=onum,
                        in1=vcum_s[:, 64 * t0:64 * t1].rearrange("p (t d) -> p t d", d=64),
                        op=OP.add)
                    rb = bass.AP(recip[:].tensor, recip[:].offset, [[4, 128], [1, 4], [0, 64]])
                    neng = nc.gpsimd if TUNE["norm_pool"] else nc.vector
                    neng.tensor_tensor(out=osl, in0=osl, in1=rb, op=OP.mult)
                    nc.sync.dma_start(out=o_d.ap()[:, 64 * t0:64 * t1],
                                      in_=out_s[:, 64 * t0:64 * t1])

                # interleaved tile order pairs small and large tiles so the
                # per-iteration engine load is roughly uniform
                order = [t for pair in zip(range(NT // 2), range(NT // 2, NT))
                         for t in pair]
                slab_done = {s: 0 for s in range(NT // 4)}
                for u in range(NT + 4):
                    if 2 <= u < NT + 2:
                        stageB(order[u - 2])
                    if u < NT:
                        stageA(order[u])
                    if u >= 4:
                        t = order[u - 4]
                        stageC(t)
                        stageOacc(t)
                        slab_done[t // 4] += 1
                        if slab_done[t // 4] == 4:
                            stageOfin(t // 4)

    nc.compile()
    return nc


def _make_runner(nc, n_cores):
    import concourse.mybir as mybir
    import jax
    from jax.sharding import Mesh, PartitionSpec
    from jax.experimental.shard_map import shard_map
    from concourse.bass2jax import install_neuronx_cc_hook, _bass_exec_p, partition_id_tensor

    install_neuronx_cc_hook()
    partition_name = nc.partition_id_tensor.name if nc.partition_id_tensor else None
    in_names, out_names, out_avals, zero_outs = [], [], [], []
    for alloc in nc.m.functions[0].allocations:
        if not isinstance(alloc, mybir.MemoryLocationSet):
            continue
        name = alloc.memorylocations[0].name
        if alloc.kind == "ExternalInput":
            if name != partition_name:
                in_names.append(name)
        elif alloc.kind == "ExternalOutput":
            shape = tuple(alloc.tensor_shape)
            dtype = mybir.dt.np(alloc.dtype)
            out_names.append(name)
            out_avals.append(jax.core.ShapedArray(shape, dtype))
            zero_outs.append(np.zeros(shape, dtype))
    n_params = len(in_names)
    n_outs = len(out_avals)
    all_in_names = list(in_names) + list(out_names)
    if partition_name is not None:
        all_in_names.append(partition_name)

    def _body(*args):
        operands = list(args)
        if partition_name is not None:
            operands.append(partition_id_tensor())
        outs = _bass_exec_p.bind(
            *operands, out_avals=tuple(out_avals), in_names=tuple(all_in_names),
            out_names=tuple(out_names), lowering_input_output_aliases=(),
            sim_require_finite=True, sim_require_nnan=True, nc=nc)
        return tuple(outs)

    devices = jax.devices()[:n_cores]
    mesh = Mesh(np.asarray(devices), ("core",))
    in_specs = (PartitionSpec("core"),) * (n_params + n_outs)
    out_specs = (PartitionSpec("core"),) * n_outs
    jitted = jax.jit(shard_map(_body, mesh=mesh, in_specs=in_specs,
                               out_specs=out_specs, check_rep=False), keep_unused=True)

    def run(in_maps):
        concat_in = [np.concatenate([np.asarray(in_maps[c][n]) for c in range(n_cores)], axis=0)
                     for n in in_names]
        concat_zeros = [np.zeros((n_cores * z.shape[0], *z.shape[1:]), z.dtype) for z in zero_outs]
        outs = jitted(*concat_in, *concat_zeros)
        import jax as _jax
        _jax.block_until_ready(outs)
        return [{name: np.asarray(outs[i]).reshape(n_cores, *out_avals[i].shape)[c]
                 for i, name in enumerate(out_names)} for c in range(n_cores)]
    return run


def _get_runner(causal: bool):
    key = bool(causal)
    if key not in _CACHE:
        nc = _build_program(key)
        _CACHE[key] = _make_runner(nc, H)
    return _CACHE[key]


def _prep_head(q2, k2, v2, causal):
    """q2,k2,v2: [N, D] fp32 for one head. Returns per-core input dict."""
    import ml_dtypes
    qT = np.concatenate([q2.T, np.ones((1, N), np.float32)], axis=0)  # [65, N]
    kT = np.concatenate([k2.T, np.ones((1, N), np.float32)], axis=0)
    # v with ones col 64 (denominator) and zero col 65 (f32r even-width pad)
    v3 = np.concatenate([v2, np.ones((N, 1), np.float32),
                         np.zeros((N, 1), np.float32)], axis=1)      # [N, 66]
    v_r = np.ascontiguousarray(
        v3.reshape(NJC, 128, 66).transpose(1, 0, 2).reshape(128, NJC * 66)
    ).astype(ml_dtypes.bfloat16)
    if causal:
        vc = np.cumsum(v2, axis=0, dtype=np.float64).astype(np.float32)
    else:
        vc = np.broadcast_to(v2.sum(axis=0, dtype=np.float64).astype(np.float32), (N, 64))
    vcum = np.ascontiguousarray(
        vc.reshape(NT, 128, 64).transpose(1, 0, 2).reshape(128, NT * 64))
    return {"qT": np.ascontiguousarray(qT), "kT": np.ascontiguousarray(kT),
            "vr": v_r, "vcum": vcum}


def kernel(q, k, v, rpe_matrix, mask):
    causal = bool(np.asarray(mask).item()) if not isinstance(mask, (int, bool)) else bool(mask)
    q = np.asarray(q, dtype=np.float32)
    k = np.asarray(k, dtype=np.float32)
    v = np.asarray(v, dtype=np.float32)
    rpe = np.asarray(rpe_matrix, dtype=np.float32)

    RPW = 2560 if causal else 4608
    if causal:
        # u in [0, N-1]: rpe_rev[u] = rpe[2N-2-u] -> rows 2N-2 .. N-1
        rpe_rev = rpe[N - 1:2 * N - 1][::-1]             # [N, 64]
    else:
        rpe_rev = rpe[::-1]                              # [2N-1, 64]
    rpeT = np.zeros((65, RPW), dtype=np.float32)
    rpeT[0:64, :rpe_rev.shape[0]] = rpe_rev.T            # row 64 stays zero

    a = np.arange(128, dtype=np.float32)[:, None]
    tt = np.arange(NT, dtype=np.float32)[None, :]
    iota = (128 * tt + a + 1.0) if causal else np.full((128, NT), float(N), np.float32)
    iota = np.ascontiguousarray(iota.astype(np.float32))

    run = _get_runner(causal)
    in_maps = []
    for h in range(H):
        m = _prep_head(q[0, h], k[0, h], v[0, h], causal)
        m["rpeT"] = rpeT
        m["iota"] = iota
        in_maps.append(m)
    results = run(in_maps)
    # o stored [128, NT*64] with o_store[a, 64t+d] = o[128t+a, d]
    outs = []
    for h in range(H):
        oh = results[h]["o"].reshape(128, NT, 64).transpose(1, 0, 2).reshape(N, 64)
        outs.append(oh)
    out = np.stack(outs)[None]  # [1, H, N, 64]
    return out.astype(np.float32)


if __name__ == "__main__":
    rng = np.random.default_rng(0)
    q = rng.standard_normal((B, H, N, D), dtype=np.float32)
    k = rng.standard_normal((B, H, N, D), dtype=np.float32)
    v = rng.standard_normal((B, H, N, D), dtype=np.float32)
    rpe = rng.standard_normal((2 * N - 1, D), dtype=np.float32)
    o = kernel(q, k, v, rpe, 1)
    print("out", o.shape, o.dtype, np.abs(o).mean())


# revision 75
# speedup vs baseline: 1.1802x; 1.0174x over previous
"""Trainium2 Bass kernel for FASTMultiHeadAttention (fastmax, Taylor-2 softmax approx
with relative positional embeddings, optional causal mask).

B=1, H=8, N=2048, D=64. One head per NeuronCore (8 cores).

Math per head (q,k,v: [N,D], rpe: [2N-1, D]):
    s[i,j]  = q_i.k_j + q_i.rpe[i-j+N-1]
    w       = 1 + s + s^2/2      (causal-masked if mask)
    out_i   = sum_j w[i,j] v_j / sum_j w[i,j]

Device algorithm (per head):
    w = ((s+1)^2 + 1)/2 on valid entries, so with t = (s+1)^2 (t=0 on masked):
      numer_i = 0.5*(sum_j t_ij v_j + vcum_i)
      denom_i = 0.5*(sum_j t_ij + (i+1))
    The +1 inside the square comes from a 65th "ones" contraction row: qT/kT
    carry a ones row (rpe a zeros row), so the content matmul yields q.k + 1
    and the full score s1 = (q.k + 1) + q.rpe needs only a tensor_tensor add.

    - content+rpe scores: K=65 f32r matmuls with bf16 PSUM output
    - rpe diagonal realignment R[a,j] = QR[a, 127-a+j]: skewed SBUF->SBUF DMA
    - s1 = S_psum + R  via bf16 tensor_tensor (DVE 2x mode)
    - causal mask: affine_select zeroes j > i on the diagonal chunk (Pool)
    - W^T via PE transposes (bf16), squared during PSUM evacuation
      (DVE tensor_tensor self-mult at 2x, or ACT Square)
    - O = sum_j t v via PE matmul with V (+ones col) stationary, K=128
    - normalize with host-precomputed vcum/iota, store [128, NT*64] row-major
"""

import sys
import os
import numpy as np

for _p in ("/opt/trn_rl_repo", "/root/.axon_site/_ro/trn_rl_repo"):
    if os.path.isdir(_p) and _p not in sys.path:
        sys.path.insert(0, _p)

B, H, N, D = 1, 8, 2048, 64
NT = N // 128            # 16 i-tiles of 128 rows
NJC = N // 128           # 16 j-chunks of 128 (for transposes / O matmul)

_CACHE = {}

# engine-assignment tuning (fractions routed to the listed engine)
TUNE = {
    "sq_act_frac": 1.0,     # (unused; squares are ACT-only, PSUM 1-input rule)
    "qr_dve_frac": 0.42,     # QR psum->sbuf copies on DVE (else ACT)
    "qr_pool_frac": 0.0,    # unused: GPSIMD cannot access PSUM
    "s1_pool_frac": 0.0,    # s1 TT chunks on Pool (else DVE)
    "gather_act_frac": 0.0, # gathers issued from ACT ring (else SP)
    "norm_pool": True,      # normalize adds on Pool (else DVE)
    "qrp_bufs": 3,
    "rrp_bufs": 6,
    "s1p_bufs": 4,
}


class _Frac:
    # weighted deterministic router: pick() True with rate `frac`
    def __init__(self, frac):
        self.f = frac
        self.acc = 0.0

    def pick(self):
        self.acc += self.f
        if self.acc >= 0.999:
            self.acc -= 1.0
            return True
        return False


def _build_program(causal: bool, reps: int = 1):
    import concourse.bass as bass
    from concourse import bacc
    import concourse.mybir as mybir
    from concourse.tile import TileContext
    from concourse.masks import make_identity

    fp32 = mybir.dt.float32
    f32r = mybir.dt.float32r
    bf16 = mybir.dt.bfloat16
    AT = mybir.ActivationFunctionType
    OP = mybir.AluOpType

    RPW = 2560 if causal else 4608   # rpe_revT padded width

    nc = bacc.Bacc("TRN2", target_bir_lowering=False, debug=False)

    qT_d = nc.dram_tensor("qT", [65, N], f32r, kind="ExternalInput")
    kT_d = nc.dram_tensor("kT", [65, N], f32r, kind="ExternalInput")
    v_d = nc.dram_tensor("vr", [128, NJC * 66], bf16, kind="ExternalInput")
    vcum_d = nc.dram_tensor("vcum", [128, NT * 64], fp32, kind="ExternalInput")
    rpe_d = nc.dram_tensor("rpeT", [65, RPW], f32r, kind="ExternalInput")
    iota_d = nc.dram_tensor("iota", [128, NT], fp32, kind="ExternalInput")
    o_d = nc.dram_tensor("o", [128, NT * 64], fp32, kind="ExternalOutput")

    def j_max(t):
        return 128 * (t + 1) if causal else N

    def u_min(t):
        return (N - 1) - 128 * t - 127

    def qr_w(t):
        return 127 + j_max(t)

    with TileContext(nc) as tc:
        with (
            tc.tile_pool(name="persist", bufs=1) as pp,
            tc.tile_pool(name="qr", bufs=TUNE["qrp_bufs"]) as qrp,
            tc.tile_pool(name="rr", bufs=TUNE["rrp_bufs"]) as rrp,
            tc.tile_pool(name="s1", bufs=TUNE["s1p_bufs"]) as s1p,
            tc.tile_pool(name="small", bufs=2) as sp,
        ):
            sq_r = _Frac(TUNE["sq_act_frac"])
            qrd_r = _Frac(TUNE["qr_dve_frac"])
            qrp_r = _Frac(TUNE["qr_pool_frac"])
            s1p_r = _Frac(TUNE["s1_pool_frac"])
            ga_r = _Frac(TUNE["gather_act_frac"])

            # ---- persistent tiles ----
            qT_s = pp.tile([65, N], f32r, name="qT_s")
            kT_s = pp.tile([65, N], f32r, name="kT_s")
            rpe_s = pp.tile([65, RPW], f32r, name="rpe_s")
            v_s = pp.tile([128, NJC * 66], bf16, name="v_s")
            vcum_s = pp.tile([128, NT * 64], fp32, name="vcum_s")
            iota_s = pp.tile([128, NT], fp32, name="iota_s")

            # chunked loads, ordered by pipeline consumption under the
            # interleaved tile order (small tile t, then tile t+8, ...)
            if causal:
                rpe_chunks = ((1920, 2176), (896, 1920), (0, 896), (2176, 2304))
            else:
                rpe_chunks = ((896, RPW), (0, 896))
            qT_chunks = ((0, 128), (1024, 1152), (128, 1024), (1152, 2048))
            kT_chunks = ((0, 256), (256, 1280), (1280, 2048))
            # fill-critical chunks on SP first (tiles 0 and 8 consume them
            # within the first two iterations); the rest on Pool SWDGE / ACT
            nc.sync.dma_start(out=qT_s[:, 0:128], in_=qT_d.ap()[:, 0:128])
            nc.scalar.dma_start(out=rpe_s[:, rpe_chunks[0][0]:rpe_chunks[0][1]],
                                in_=rpe_d.ap()[:, rpe_chunks[0][0]:rpe_chunks[0][1]])
            nc.sync.dma_start(out=qT_s[:, 1024:1152], in_=qT_d.ap()[:, 1024:1152])
            nc.sync.dma_start(out=kT_s[:, 0:256], in_=kT_d.ap()[:, 0:256])
            nc.sync.dma_start(out=rpe_s[:, rpe_chunks[1][0]:rpe_chunks[1][1]],
                              in_=rpe_d.ap()[:, rpe_chunks[1][0]:rpe_chunks[1][1]])
            def bulk_loads_a():
                # consumed first: qT for tiles 1/9, kT body
                for c0, c1 in qT_chunks[2:]:
                    nc.sync.dma_start(out=qT_s[:, c0:c1], in_=qT_d.ap()[:, c0:c1])
                for c0, c1 in kT_chunks[1:]:
                    nc.sync.dma_start(out=kT_s[:, c0:c1], in_=kT_d.ap()[:, c0:c1])
                nc.gpsimd.dma_start(out=v_s[:], in_=v_d.ap())

            def bulk_loads_b():
                for c0, c1 in rpe_chunks[2:]:
                    nc.sync.dma_start(out=rpe_s[:, c0:c1], in_=rpe_d.ap()[:, c0:c1])
                nc.gpsimd.dma_start(out=vcum_s[:], in_=vcum_d.ap())
                nc.gpsimd.dma_start(out=iota_s[:], in_=iota_d.ap())

            bulk_loads_a()
            bulk_loads_b()

            ident = pp.tile([128, 128], bf16, name="ident")
            make_identity(nc, ident[:])
            ident66_f = pp.tile([66, 66], fp32, name="ident66_f")
            make_identity(nc, ident66_f[:])
            ident66_r = pp.tile([66, 66], f32r, name="ident66_r")
            nc.vector.tensor_copy(ident66_r[:], ident66_f[:])
            ident66 = ident66_r[:]

            # W^T storage, triangular-packed by groups of 4 j-chunks when causal:
            # group g0 stores only i >= 128*g0 (width Wg = N - 128*g0).
            def wt_imin(jc):
                return 128 * (4 * (jc // 4)) if causal else 0

            def wt_w(jc):
                return N - wt_imin(jc)

            _wt_base = {}
            _off = 0
            for _jc in range(NJC):
                _wt_base[_jc] = _off
                _off += wt_w(_jc)
            WTW = _off
            wt_all = pp.tile([128, WTW], bf16, name="wt_all")

            out_s = pp.tile([128, NT * 64], fp32, name="out_s")

            for _rep in range(reps):
              with (
                  tc.tile_pool(name="qr_ps", bufs=2, space="PSUM") as qrps,
                  tc.tile_pool(name="s_ps", bufs=2, space="PSUM") as sps,
                  tc.tile_pool(name="tr_ps", bufs=2, space="PSUM") as trp,
              ):
                live = {}

                def mm65(out_ps, t, src, c0, mw):
                    i0 = 128 * t
                    nc.tensor.matmul(out_ps, qT_s[:, i0:i0 + 128],
                                     src[:, c0:c0 + mw],
                                     start=True, stop=True, tile_position=(0, 0))

                def stageA(t):
                    # rpe projection QR (K=65, zero row kills the ones term),
                    # fp32 psum, ACT evac to bf16, then diagonal gather of R
                    w = qr_w(t)
                    um = u_min(t)
                    qrbuf = qrp.tile([128, 2560 if causal else 2304], bf16, name="qrbuf")
                    for b0 in range(0, w, 1024):
                        bw = min(1024, w - b0)
                        qr_ps = qrps.tile([128, 1024], fp32, name="qr_ps")
                        for h0 in range(0, bw, 512):
                            hw = min(512, bw - h0)
                            mw = max(256, (hw + 1) & ~1)  # f32r ISA: even, >= 256
                            mm65(qr_ps[:, h0:h0 + mw], t, rpe_s, um + b0 + h0, mw)
                        if qrd_r.pick():
                            nc.vector.tensor_copy(qrbuf[:, b0:b0 + bw], qr_ps[:, 0:bw])
                        elif qrp_r.pick():
                            nc.gpsimd.tensor_copy(qrbuf[:, b0:b0 + bw], qr_ps[:, 0:bw])
                        else:
                            nc.scalar.activation(qrbuf[:, b0:b0 + bw], qr_ps[:, 0:bw],
                                                 AT.Copy, bias=0.0, scale=1.0)
                    # diagonal gather R[a, j] = qrbuf[a, 127 - a + j]
                    QW = qrbuf[:].tensor.shape[1]
                    R_row = rrp.tile([128, N], bf16, name="R_row")
                    diag = bass.AP(qrbuf[:].tensor, qrbuf[:].offset + 127,
                                   [[QW - 1, 128], [1, j_max(t)]])
                    eng = nc.scalar if ga_r.pick() else nc.sync
                    eng.dma_start(out=R_row[:, 0:j_max(t)], in_=diag)
                    live[("A", t)] = R_row

                def stageB(t):
                    # content scores (K=65 with ones row -> q.k + 1), bf16 psum,
                    # s1 = S + R via DVE tensor_tensor (2x), causal mask on diag
                    i0 = 128 * t
                    jm = j_max(t)
                    R_row = live.pop(("A", t))
                    s1_row = s1p.tile([128, N], bf16, name="s1_row", tag="s1_row")
                    for jb in range(0, jm, 512):
                        cw = min(512, jm - jb)
                        s_ps = sps.tile([128, 512], fp32, name="s_ps")
                        mw = max(256, (cw + 1) & ~1)
                        mm65(s_ps[:, 0:mw], t, kT_s, jb, mw)
                        teng = nc.gpsimd if s1p_r.pick() else nc.vector
                        teng.tensor_tensor(
                            out=s1_row[:, jb:jb + cw], in0=s_ps[:, 0:cw],
                            in1=R_row[:, jb:jb + cw], op=OP.add)
                    s1_diag = None
                    if causal:
                        # masked diagonal chunk goes to its own tile so the mask
                        # doesn't gate the other chunks' transposes
                        s1_diag = s1p.tile([128, 128], bf16, name="s1_diag", tag="s1_diag")
                        nc.gpsimd.affine_select(
                            out=s1_diag[:], in_=s1_row[:, i0:i0 + 128],
                            compare_op=OP.is_ge, fill=0.0,
                            base=0, channel_multiplier=1, pattern=[[-1, 128]])
                    live[("B", t)] = (s1_diag, s1_row)

                def stageC(t):
                    # transpose s1 chunks, square during PSUM evacuation -> wt_all
                    i0 = 128 * t
                    s1_diag, s1_row = live.pop(("B", t))
                    njc = (j_max(t) + 127) // 128
                    for g0 in range(0, njc, 4):
                        gn = min(4, njc - g0)
                        tr_ps = trp.tile([128, 512], bf16, name="tr_ps")
                        for g in range(gn):
                            jc = g0 + g
                            src_chunk = (s1_diag[:] if (causal and jc == t)
                                         else s1_row[:, 128 * jc:128 * (jc + 1)])
                            nc.tensor.transpose(tr_ps[:, 128 * g:128 * (g + 1)],
                                                src_chunk, ident[:])
                        dst = bass.AP(wt_all[:].tensor,
                                      wt_all[:].offset + _wt_base[g0] + (i0 - wt_imin(g0)),
                                      [[WTW, 128], [wt_w(g0), gn], [1, 128]])
                        srcap = tr_ps[:, 0:128 * gn].rearrange("p (g c) -> p g c", g=gn)
                        # PSUM allows only one tensor input per instruction, so
                        # the square must be ACT's single-input Square
                        nc.scalar.activation(dst, srcap, AT.Square, bias=0.0, scale=1.0)

                def stageOacc(t):
                    # accumulate O for i-range [128t, 128t+128) over its j-chunks
                    # right after stageC(t) wrote those W^T columns; transient
                    # psum partial, evacuated straight to the slab SBUF tile
                    s = t // 4
                    if ("O", s) not in live:
                        live[("O", s)] = sp.tile([66, 512], f32r, name="oT_s", tag="oT_s")
                    oT_s = live[("O", s)]
                    c0 = 128 * (t % 4)
                    o_ps = sps.tile([66, 128], fp32, name="s_ps")
                    jc_hi = t + 1 if causal else NJC
                    for jc in range(jc_hi):
                        rhs = bass.AP(wt_all[:].tensor,
                                      wt_all[:].offset + _wt_base[jc] + (128 * t - wt_imin(jc)),
                                      [[WTW, 128], [1, 128]])
                        nc.tensor.matmul(o_ps[:, 0:128],
                                         v_s[:, 66 * jc:66 * (jc + 1)], rhs,
                                         start=(jc == 0), stop=(jc == jc_hi - 1))
                    nc.scalar.activation(oT_s[:, c0:c0 + 128], o_ps[:, 0:128],
                                         AT.Copy, bias=0.0, scale=1.0)

                def stageOfin(s):
                    # back-transpose + normalize + store slab s
                    t0, t1 = 4 * s, 4 * s + 4
                    oT_s = live.pop(("O", s))
                    ob_ps = qrps.tile([128, 264], f32r, name="qr_ps")
                    for g in range(4):
                        nc.tensor.transpose(ob_ps[:, 66 * g:66 * (g + 1)],
                                            oT_s[:, 128 * g:128 * (g + 1)],
                                            ident66)
                    # normalize straight from the back-transpose psum (one PSUM
                    # input per instruction is legal on DVE)
                    obf = ob_ps[:].bitcast(fp32)
                    dtot = sp.tile([128, 4], fp32, name="dtot", tag="dtot")
                    dcol = bass.AP(obf.tensor, obf.offset + 64, [[264, 128], [66, 4]])
                    nc.vector.tensor_tensor(out=dtot[:], in0=dcol, in1=iota_s[:, t0:t1], op=OP.add)
                    recip = sp.tile([128, 4], fp32, name="recip", tag="recip")
                    nc.vector.reciprocal(recip[:], dtot[:])
                    onum = bass.AP(obf.tensor, obf.offset, [[264, 128], [66, 4], [1, 64]])
                    osl = out_s[:, 64 * t0:64 * t1].rearrange("p (t d) -> p t d", d=64)
                    nc.vector.tensor_tensor(
                        out=osl, in0=onum,
                        in1=vcum_s[:, 64 * t0:64 * t1].rearrange("p (t d) -> p t d", d=64),
                        op=OP.add)
                    rb = bass.AP(recip[:].tensor, recip[:].offset, [[4, 128], [1, 4], [0, 64]])
                    neng = nc.gpsimd if TUNE["norm_pool"] else nc.vector
                    neng.tensor_tensor(out=osl, in0=osl, in1=rb, op=OP.mult)
                    nc.sync.dma_start(out=o_d.ap()[:, 64 * t0:64 * t1],
                                      in_=out_s[:, 64 * t0:64 * t1])

                # interleaved tile order pairs small and large tiles so the
                # per-iteration engine load is roughly uniform
                order = [t for pair in zip(range(NT // 2), range(NT // 2, NT))
                         for t in pair]
                slab_done = {s: 0 for s in range(NT // 4)}
                for u in range(NT + 4):
                    if 2 <= u < NT + 2:
                        stageB(order[u - 2])
                    if u < NT:
                        stageA(order[u])
                    if u >= 4:
                        t = order[u - 4]
                        stageC(t)
                        stageOacc(t)
                        slab_done[t // 4] += 1
                        if slab_done[t // 4] == 4:
                            stageOfin(t // 4)

    nc.compile()
    return nc


def _make_runner(nc, n_cores):
    import concourse.mybir as mybir
    import jax
    from jax.sharding import Mesh, PartitionSpec
    from jax.experimental.shard_map import shard_map
    from concourse.bass2jax import install_neuronx_cc_hook, _bass_exec_p, partition_id_tensor

    install_neuronx_cc_hook()
    partition_name = nc.partition_id_tensor.name if nc.partition_id_tensor else None
    in_names, out_names, out_avals, zero_outs = [], [], [], []
    for alloc in nc.m.functions[0].allocations:
        if not isinstance(alloc, mybir.MemoryLocationSet):
            continue
        name = alloc.memorylocations[0].name
        if alloc.kind == "ExternalInput":
            if name != partition_name:
                in_names.append(name)
        elif alloc.kind == "ExternalOutput":
            shape = tuple(alloc.tensor_shape)
            dtype = mybir.dt.np(alloc.dtype)
            out_names.append(name)
            out_avals.append(jax.core.ShapedArray(shape, dtype))
            zero_outs.append(np.zeros(shape, dtype))
    n_params = len(in_names)
    n_outs = len(out_avals)
    all_in_names = list(in_names) + list(out_names)
    if partition_name is not None:
        all_in_names.append(partition_name)

    def _body(*args):
        operands = list(args)
        if partition_name is not None:
            operands.append(partition_id_tensor())
        outs = _bass_exec_p.bind(
            *operands, out_avals=tuple(out_avals), in_names=tuple(all_in_names),
            out_names=tuple(out_names), lowering_input_output_aliases=(),
            sim_require_finite=True, sim_require_nnan=True, nc=nc)
        return tuple(outs)

    devices = jax.devices()[:n_cores]
    mesh = Mesh(np.asarray(devices), ("core",))
    in_specs = (PartitionSpec("core"),) * (n_params + n_outs)
    out_specs = (PartitionSpec("core"),) * n_outs
    jitted = jax.jit(shard_map(_body, mesh=mesh, in_specs=in_specs,
                               out_specs=out_specs, check_rep=False), keep_unused=True)

    def run(in_maps):
        concat_in = [np.concatenate([np.asarray(in_maps[c][n]) for c in range(n_cores)], axis=0)
                     for n in in_names]
        concat_zeros = [np.zeros((n_cores * z.shape[0], *z.shape[1:]), z.dtype) for z in zero_outs]
        outs = jitted(*concat_in, *concat_zeros)
        import jax as _jax
        _jax.block_until_ready(outs)
        return [{name: np.asarray(outs[i]).reshape(n_cores, *out_avals[i].shape)[c]
                 for i, name in enumerate(out_names)} for c in range(n_cores)]
    return run


def _get_runner(causal: bool):
    key = bool(causal)
    if key not in _CACHE:
        nc = _build_program(key)
        _CACHE[key] = _make_runner(nc, H)
    return _CACHE[key]


def _prep_head(q2, k2, v2, causal):
    """q2,k2,v2: [N, D] fp32 for one head. Returns per-core input dict."""
    import ml_dtypes
    qT = np.concatenate([q2.T, np.ones((1, N), np.float32)], axis=0)  # [65, N]
    kT = np.concatenate([k2.T, np.ones((1, N), np.float32)], axis=0)
    # v with ones col 64 (denominator) and zero col 65 (f32r even-width pad)
    v3 = np.concatenate([v2, np.ones((N, 1), np.float32),
                         np.zeros((N, 1), np.float32)], axis=1)      # [N, 66]
    v_r = np.ascontiguousarray(
        v3.reshape(NJC, 128, 66).transpose(1, 0, 2).reshape(128, NJC * 66)
    ).astype(ml_dtypes.bfloat16)
    if causal:
        vc = np.cumsum(v2, axis=0, dtype=np.float64).astype(np.float32)
    else:
        vc = np.broadcast_to(v2.sum(axis=0, dtype=np.float64).astype(np.float32), (N, 64))
    vcum = np.ascontiguousarray(
        vc.reshape(NT, 128, 64).transpose(1, 0, 2).reshape(128, NT * 64))
    return {"qT": np.ascontiguousarray(qT), "kT": np.ascontiguousarray(kT),
            "vr": v_r, "vcum": vcum}


def kernel(q, k, v, rpe_matrix, mask):
    causal = bool(np.asarray(mask).item()) if not isinstance(mask, (int, bool)) else bool(mask)
    q = np.asarray(q, dtype=np.float32)
    k = np.asarray(k, dtype=np.float32)
    v = np.asarray(v, dtype=np.float32)
    rpe = np.asarray(rpe_matrix, dtype=np.float32)

    RPW = 2560 if causal else 4608
    if causal:
        # u in [0, N-1]: rpe_rev[u] = rpe[2N-2-u] -> rows 2N-2 .. N-1
        rpe_rev = rpe[N - 1:2 * N - 1][::-1]             # [N, 64]
    else:
        rpe_rev = rpe[::-1]                              # [2N-1, 64]
    rpeT = np.zeros((65, RPW), dtype=np.float32)
    rpeT[0:64, :rpe_rev.shape[0]] = rpe_rev.T            # row 64 stays zero

    a = np.arange(128, dtype=np.float32)[:, None]
    tt = np.arange(NT, dtype=np.float32)[None, :]
    iota = (128 * tt + a + 1.0) if causal else np.full((128, NT), float(N), np.float32)
    iota = np.ascontiguousarray(iota.astype(np.float32))

    run = _get_runner(causal)
    in_maps = []
    for h in range(H):
        m = _prep_head(q[0, h], k[0, h], v[0, h], causal)
        m["rpeT"] = rpeT
        m["iota"] = iota
        in_maps.append(m)
    results = run(in_maps)
    # o stored [128, NT*64] with o_store[a, 64t+d] = o[128t+a, d]
    outs = []
    for h in range(H):
        oh = results[h]["o"].reshape(128, NT, 64).transpose(1, 0, 2).reshape(N, 64)
        outs.append(oh)
    out = np.stack(outs)[None]  # [1, H, N, 64]
    return out.astype(np.float32)


if __name__ == "__main__":
    rng = np.random.default_rng(0)
    q = rng.standard_normal((B, H, N, D), dtype=np.float32)
    k = rng.standard_normal((B, H, N, D), dtype=np.float32)
    v = rng.standard_normal((B, H, N, D), dtype=np.float32)
    rpe = rng.standard_normal((2 * N - 1, D), dtype=np.float32)
    o = kernel(q, k, v, rpe, 1)
    print("out", o.shape, o.dtype, np.abs(o).mean())


# revision 76
# speedup vs baseline: 1.1818x; 1.0013x over previous
"""Trainium2 Bass kernel for FASTMultiHeadAttention (fastmax, Taylor-2 softmax approx
with relative positional embeddings, optional causal mask).

B=1, H=8, N=2048, D=64. One head per NeuronCore (8 cores).

Math per head (q,k,v: [N,D], rpe: [2N-1, D]):
    s[i,j]  = q_i.k_j + q_i.rpe[i-j+N-1]
    w       = 1 + s + s^2/2      (causal-masked if mask)
    out_i   = sum_j w[i,j] v_j / sum_j w[i,j]

Device algorithm (per head):
    w = ((s+1)^2 + 1)/2 on valid entries, so with t = (s+1)^2 (t=0 on masked):
      numer_i = 0.5*(sum_j t_ij v_j + vcum_i)
      denom_i = 0.5*(sum_j t_ij + (i+1))
    The +1 inside the square comes from a 65th "ones" contraction row: qT/kT
    carry a ones row (rpe a zeros row), so the content matmul yields q.k + 1
    and the full score s1 = (q.k + 1) + q.rpe needs only a tensor_tensor add.

    - content+rpe scores: K=65 f32r matmuls (fp32 PSUM)
    - rpe diagonal realignment R[a,j] = QR[a, 127-a+j]: skewed SBUF->SBUF DMA
    - s1 = S_psum + R  via tensor_tensor on DVE (single PSUM input)
    - causal mask: affine_select zeroes j > i on the diagonal chunk (Pool)
    - W^T via PE transposes (bf16), squared during PSUM evacuation
      (ACT Square; hardware allows only one PSUM input per instruction)
    - O = sum_j t v via PE matmul with V (+ones col) stationary, K=128
    - normalize with host-precomputed vcum/iota, store [128, NT*64] row-major
"""

import sys
import os
import numpy as np

for _p in ("/opt/trn_rl_repo", "/root/.axon_site/_ro/trn_rl_repo"):
    if os.path.isdir(_p) and _p not in sys.path:
        sys.path.insert(0, _p)

B, H, N, D = 1, 8, 2048, 64
NT = N // 128            # 16 i-tiles of 128 rows
NJC = N // 128           # 16 j-chunks of 128 (for transposes / O matmul)

_CACHE = {}

# engine-assignment tuning (fractions routed to the listed engine)
TUNE = {
    "sq_act_frac": 1.0,     # (unused; squares are ACT-only, PSUM 1-input rule)
    "qr_dve_frac": 0.42,     # QR psum->sbuf copies on DVE (else ACT)
    "qr_pool_frac": 0.0,    # unused: GPSIMD cannot access PSUM
    "s1_pool_frac": 0.0,    # s1 TT chunks on Pool (else DVE)
    "gather_act_frac": 0.0, # gathers issued from ACT ring (else SP)
    "norm_pool": True,      # normalize adds on Pool (else DVE)
    "qrp_bufs": 4,
    "rrp_bufs": 6,
    "s1p_bufs": 4,
}


class _Frac:
    # weighted deterministic router: pick() True with rate `frac`
    def __init__(self, frac):
        self.f = frac
        self.acc = 0.0

    def pick(self):
        self.acc += self.f
        if self.acc >= 0.999:
            self.acc -= 1.0
            return True
        return False


def _build_program(causal: bool, reps: int = 1):
    import concourse.bass as bass
    from concourse import bacc
    import concourse.mybir as mybir
    from concourse.tile import TileContext
    from concourse.masks import make_identity

    fp32 = mybir.dt.float32
    f32r = mybir.dt.float32r
    bf16 = mybir.dt.bfloat16
    AT = mybir.ActivationFunctionType
    OP = mybir.AluOpType

    RPW = 2560 if causal else 4608   # rpe_revT padded width

    nc = bacc.Bacc("TRN2", target_bir_lowering=False, debug=False)

    qT_d = nc.dram_tensor("qT", [65, N], f32r, kind="ExternalInput")
    kT_d = nc.dram_tensor("kT", [65, N], f32r, kind="ExternalInput")
    v_d = nc.dram_tensor("vr", [128, NJC * 66], bf16, kind="ExternalInput")
    vcum_d = nc.dram_tensor("vcum", [128, NT * 64], fp32, kind="ExternalInput")
    rpe_d = nc.dram_tensor("rpeT", [65, RPW], f32r, kind="ExternalInput")
    iota_d = nc.dram_tensor("iota", [128, NT], fp32, kind="ExternalInput")
    o_d = nc.dram_tensor("o", [128, NT * 64], fp32, kind="ExternalOutput")

    def j_max(t):
        return 128 * (t + 1) if causal else N

    def u_min(t):
        return (N - 1) - 128 * t - 127

    def qr_w(t):
        return 127 + j_max(t)

    with TileContext(nc) as tc:
        with (
            tc.tile_pool(name="persist", bufs=1) as pp,
            tc.tile_pool(name="qr", bufs=TUNE["qrp_bufs"]) as qrp,
            tc.tile_pool(name="rr", bufs=TUNE["rrp_bufs"]) as rrp,
            tc.tile_pool(name="s1", bufs=TUNE["s1p_bufs"]) as s1p,
            tc.tile_pool(name="small", bufs=2) as sp,
        ):
            sq_r = _Frac(TUNE["sq_act_frac"])
            qrd_r = _Frac(TUNE["qr_dve_frac"])
            qrp_r = _Frac(TUNE["qr_pool_frac"])
            s1p_r = _Frac(TUNE["s1_pool_frac"])
            ga_r = _Frac(TUNE["gather_act_frac"])

            # ---- persistent tiles ----
            qT_s = pp.tile([65, N], f32r, name="qT_s")
            kT_s = pp.tile([65, N], f32r, name="kT_s")
            rpe_s = pp.tile([65, RPW], f32r, name="rpe_s")
            v_s = pp.tile([128, NJC * 66], bf16, name="v_s")
            vcum_s = pp.tile([128, NT * 64], fp32, name="vcum_s")
            iota_s = pp.tile([128, NT], fp32, name="iota_s")

            # chunked loads, ordered by pipeline consumption under the
            # interleaved tile order (small tile t, then tile t+8, ...)
            if causal:
                rpe_chunks = ((1920, 2176), (896, 1920), (0, 896), (2176, 2304))
            else:
                rpe_chunks = ((896, RPW), (0, 896))
            qT_chunks = ((0, 128), (1024, 1152), (128, 1024), (1152, 2048))
            kT_chunks = ((0, 256), (256, 1280), (1280, 2048))
            # fill-critical chunks on SP first (tiles 0 and 8 consume them
            # within the first two iterations); the rest on Pool SWDGE / ACT
            nc.sync.dma_start(out=qT_s[:, 0:128], in_=qT_d.ap()[:, 0:128])
            nc.scalar.dma_start(out=rpe_s[:, rpe_chunks[0][0]:rpe_chunks[0][1]],
                                in_=rpe_d.ap()[:, rpe_chunks[0][0]:rpe_chunks[0][1]])
            nc.sync.dma_start(out=qT_s[:, 1024:1152], in_=qT_d.ap()[:, 1024:1152])
            nc.sync.dma_start(out=kT_s[:, 0:256], in_=kT_d.ap()[:, 0:256])
            nc.sync.dma_start(out=rpe_s[:, rpe_chunks[1][0]:rpe_chunks[1][1]],
                              in_=rpe_d.ap()[:, rpe_chunks[1][0]:rpe_chunks[1][1]])
            def bulk_loads_a():
                # consumed first: qT for tiles 1/9, kT body
                for c0, c1 in qT_chunks[2:]:
                    nc.sync.dma_start(out=qT_s[:, c0:c1], in_=qT_d.ap()[:, c0:c1])
                for c0, c1 in kT_chunks[1:]:
                    nc.sync.dma_start(out=kT_s[:, c0:c1], in_=kT_d.ap()[:, c0:c1])
                nc.gpsimd.dma_start(out=v_s[:], in_=v_d.ap())

            def bulk_loads_b():
                for c0, c1 in rpe_chunks[2:]:
                    nc.sync.dma_start(out=rpe_s[:, c0:c1], in_=rpe_d.ap()[:, c0:c1])
                nc.gpsimd.dma_start(out=vcum_s[:], in_=vcum_d.ap())
                nc.gpsimd.dma_start(out=iota_s[:], in_=iota_d.ap())

            bulk_loads_a()
            bulk_loads_b()

            ident = pp.tile([128, 128], bf16, name="ident")
            make_identity(nc, ident[:])
            ident66_f = pp.tile([66, 66], fp32, name="ident66_f")
            make_identity(nc, ident66_f[:])
            ident66_r = pp.tile([66, 66], f32r, name="ident66_r")
            nc.vector.tensor_copy(ident66_r[:], ident66_f[:])
            ident66 = ident66_r[:]

            # W^T storage, triangular-packed by groups of 4 j-chunks when causal:
            # group g0 stores only i >= 128*g0 (width Wg = N - 128*g0).
            def wt_imin(jc):
                return 128 * (4 * (jc // 4)) if causal else 0

            def wt_w(jc):
                return N - wt_imin(jc)

            _wt_base = {}
            _off = 0
            for _jc in range(NJC):
                _wt_base[_jc] = _off
                _off += wt_w(_jc)
            WTW = _off
            wt_all = pp.tile([128, WTW], bf16, name="wt_all")

            out_s = pp.tile([128, NT * 64], fp32, name="out_s")

            for _rep in range(reps):
              with (
                  tc.tile_pool(name="qr_ps", bufs=2, space="PSUM") as qrps,
                  tc.tile_pool(name="s_ps", bufs=2, space="PSUM") as sps,
                  tc.tile_pool(name="tr_ps", bufs=2, space="PSUM") as trp,
              ):
                live = {}

                def mm65(out_ps, t, src, c0, mw):
                    i0 = 128 * t
                    nc.tensor.matmul(out_ps, qT_s[:, i0:i0 + 128],
                                     src[:, c0:c0 + mw],
                                     start=True, stop=True, tile_position=(0, 0))

                def stageA(t):
                    # rpe projection QR (K=65, zero row kills the ones term),
                    # fp32 psum, ACT evac to bf16, then diagonal gather of R
                    w = qr_w(t)
                    um = u_min(t)
                    qrbuf = qrp.tile([128, 2560 if causal else 2304], bf16, name="qrbuf")
                    for b0 in range(0, w, 1024):
                        bw = min(1024, w - b0)
                        qr_ps = qrps.tile([128, 1024], fp32, name="qr_ps")
                        for h0 in range(0, bw, 512):
                            hw = min(512, bw - h0)
                            mw = max(256, (hw + 1) & ~1)  # f32r ISA: even, >= 256
                            mm65(qr_ps[:, h0:h0 + mw], t, rpe_s, um + b0 + h0, mw)
                        if qrd_r.pick():
                            nc.vector.tensor_copy(qrbuf[:, b0:b0 + bw], qr_ps[:, 0:bw])
                        elif qrp_r.pick():
                            nc.gpsimd.tensor_copy(qrbuf[:, b0:b0 + bw], qr_ps[:, 0:bw])
                        else:
                            nc.scalar.activation(qrbuf[:, b0:b0 + bw], qr_ps[:, 0:bw],
                                                 AT.Copy, bias=0.0, scale=1.0)
                    # diagonal gather R[a, j] = qrbuf[a, 127 - a + j]
                    QW = qrbuf[:].tensor.shape[1]
                    R_row = rrp.tile([128, N], bf16, name="R_row")
                    diag = bass.AP(qrbuf[:].tensor, qrbuf[:].offset + 127,
                                   [[QW - 1, 128], [1, j_max(t)]])
                    eng = nc.scalar if ga_r.pick() else nc.sync
                    eng.dma_start(out=R_row[:, 0:j_max(t)], in_=diag)
                    live[("A", t)] = R_row

                def stageB(t):
                    # content scores (K=65 with ones row -> q.k + 1), bf16 psum,
                    # s1 = S + R via DVE tensor_tensor (2x), causal mask on diag
                    i0 = 128 * t
                    jm = j_max(t)
                    R_row = live.pop(("A", t))
                    s1_row = s1p.tile([128, N], bf16, name="s1_row", tag="s1_row")
                    for jb in range(0, jm, 512):
                        cw = min(512, jm - jb)
                        s_ps = sps.tile([128, 512], fp32, name="s_ps")
                        mw = max(256, (cw + 1) & ~1)
                        mm65(s_ps[:, 0:mw], t, kT_s, jb, mw)
                        teng = nc.gpsimd if s1p_r.pick() else nc.vector
                        teng.tensor_tensor(
                            out=s1_row[:, jb:jb + cw], in0=s_ps[:, 0:cw],
                            in1=R_row[:, jb:jb + cw], op=OP.add)
                    s1_diag = None
                    if causal:
                        # masked diagonal chunk goes to its own tile so the mask
                        # doesn't gate the other chunks' transposes
                        s1_diag = s1p.tile([128, 128], bf16, name="s1_diag", tag="s1_diag")
                        nc.gpsimd.affine_select(
                            out=s1_diag[:], in_=s1_row[:, i0:i0 + 128],
                            compare_op=OP.is_ge, fill=0.0,
                            base=0, channel_multiplier=1, pattern=[[-1, 128]])
                    live[("B", t)] = (s1_diag, s1_row)

                def stageC(t):
                    # transpose s1 chunks, square during PSUM evacuation -> wt_all
                    i0 = 128 * t
                    s1_diag, s1_row = live.pop(("B", t))
                    njc = (j_max(t) + 127) // 128
                    for g0 in range(0, njc, 4):
                        gn = min(4, njc - g0)
                        tr_ps = trp.tile([128, 512], bf16, name="tr_ps")
                        for g in range(gn):
                            jc = g0 + g
                            src_chunk = (s1_diag[:] if (causal and jc == t)
                                         else s1_row[:, 128 * jc:128 * (jc + 1)])
                            nc.tensor.transpose(tr_ps[:, 128 * g:128 * (g + 1)],
                                                src_chunk, ident[:])
                        dst = bass.AP(wt_all[:].tensor,
                                      wt_all[:].offset + _wt_base[g0] + (i0 - wt_imin(g0)),
                                      [[WTW, 128], [wt_w(g0), gn], [1, 128]])
                        srcap = tr_ps[:, 0:128 * gn].rearrange("p (g c) -> p g c", g=gn)
                        # PSUM allows only one tensor input per instruction, so
                        # the square must be ACT's single-input Square
                        nc.scalar.activation(dst, srcap, AT.Square, bias=0.0, scale=1.0)

                def stageOacc(t):
                    # accumulate O for i-range [128t, 128t+128) over its j-chunks
                    # right after stageC(t) wrote those W^T columns; transient
                    # psum partial, evacuated straight to the slab SBUF tile
                    s = t // 4
                    if ("O", s) not in live:
                        live[("O", s)] = sp.tile([66, 512], f32r, name="oT_s", tag="oT_s")
                    oT_s = live[("O", s)]
                    c0 = 128 * (t % 4)
                    o_ps = sps.tile([66, 128], fp32, name="s_ps")
                    jc_hi = t + 1 if causal else NJC
                    for jc in range(jc_hi):
                        rhs = bass.AP(wt_all[:].tensor,
                                      wt_all[:].offset + _wt_base[jc] + (128 * t - wt_imin(jc)),
                                      [[WTW, 128], [1, 128]])
                        nc.tensor.matmul(o_ps[:, 0:128],
                                         v_s[:, 66 * jc:66 * (jc + 1)], rhs,
                                         start=(jc == 0), stop=(jc == jc_hi - 1))
                    nc.scalar.activation(oT_s[:, c0:c0 + 128], o_ps[:, 0:128],
                                         AT.Copy, bias=0.0, scale=1.0)

                def stageOfin(s):
                    # back-transpose + normalize + store slab s
                    t0, t1 = 4 * s, 4 * s + 4
                    oT_s = live.pop(("O", s))
                    ob_ps = qrps.tile([128, 264], f32r, name="qr_ps")
                    for g in range(4):
                        nc.tensor.transpose(ob_ps[:, 66 * g:66 * (g + 1)],
                                            oT_s[:, 128 * g:128 * (g + 1)],
                                            ident66)
                    # normalize straight from the back-transpose psum (one PSUM
                    # input per instruction is legal on DVE)
                    obf = ob_ps[:].bitcast(fp32)
                    dtot = sp.tile([128, 4], fp32, name="dtot", tag="dtot")
                    dcol = bass.AP(obf.tensor, obf.offset + 64, [[264, 128], [66, 4]])
                    nc.vector.tensor_tensor(out=dtot[:], in0=dcol, in1=iota_s[:, t0:t1], op=OP.add)
                    recip = sp.tile([128, 4], fp32, name="recip", tag="recip")
                    nc.vector.reciprocal(recip[:], dtot[:])
                    onum = bass.AP(obf.tensor, obf.offset, [[264, 128], [66, 4], [1, 64]])
                    osl = out_s[:, 64 * t0:64 * t1].rearrange("p (t d) -> p t d", d=64)
                    nc.vector.tensor_tensor(
                        out=osl, in0=onum,
                        in1=vcum_s[:, 64 * t0:64 * t1].rearrange("p (t d) -> p t d", d=64),
                        op=OP.add)
                    rb = bass.AP(recip[:].tensor, recip[:].offset, [[4, 128], [1, 4], [0, 64]])
                    neng = nc.gpsimd if TUNE["norm_pool"] else nc.vector
                    neng.tensor_tensor(out=osl, in0=osl, in1=rb, op=OP.mult)
                    nc.sync.dma_start(out=o_d.ap()[:, 64 * t0:64 * t1],
                                      in_=out_s[:, 64 * t0:64 * t1])

                # interleaved tile order pairs small and large tiles so the
                # per-iteration engine load is roughly uniform
                order = [t for pair in zip(range(NT // 2), range(NT // 2, NT))
                         for t in pair]
                slab_done = {s: 0 for s in range(NT // 4)}
                for u in range(NT + 4):
                    if 2 <= u < NT + 2:
                        stageB(order[u - 2])
                    if u < NT:
                        stageA(order[u])
                    if u >= 4:
                        t = order[u - 4]
                        stageC(t)
                        stageOacc(t)
                        slab_done[t // 4] += 1
                        if slab_done[t // 4] == 4:
                            stageOfin(t // 4)

    nc.compile()
    return nc


def _make_runner(nc, n_cores):
    import concourse.mybir as mybir
    import jax
    from jax.sharding import Mesh, PartitionSpec
    from jax.experimental.shard_map import shard_map
    from concourse.bass2jax import install_neuronx_cc_hook, _bass_exec_p, partition_id_tensor

    install_neuronx_cc_hook()
    partition_name = nc.partition_id_tensor.name if nc.partition_id_tensor else None
    in_names, out_names, out_avals, zero_outs = [], [], [], []
    for alloc in nc.m.functions[0].allocations:
        if not isinstance(alloc, mybir.MemoryLocationSet):
            continue
        name = alloc.memorylocations[0].name
        if alloc.kind == "ExternalInput":
            if name != partition_name:
                in_names.append(name)
        elif alloc.kind == "ExternalOutput":
            shape = tuple(alloc.tensor_shape)
            dtype = mybir.dt.np(alloc.dtype)
            out_names.append(name)
            out_avals.append(jax.core.ShapedArray(shape, dtype))
            zero_outs.append(np.zeros(shape, dtype))
    n_params = len(in_names)
    n_outs = len(out_avals)
    all_in_names = list(in_names) + list(out_names)
    if partition_name is not None:
        all_in_names.append(partition_name)

    def _body(*args):
        operands = list(args)
        if partition_name is not None:
            operands.append(partition_id_tensor())
        outs = _bass_exec_p.bind(
            *operands, out_avals=tuple(out_avals), in_names=tuple(all_in_names),
            out_names=tuple(out_names), lowering_input_output_aliases=(),
            sim_require_finite=True, sim_require_nnan=True, nc=nc)
        return tuple(outs)

    devices = jax.devices()[:n_cores]
    mesh = Mesh(np.asarray(devices), ("core",))
    in_specs = (PartitionSpec("core"),) * (n_params + n_outs)
    out_specs = (PartitionSpec("core"),) * n_outs
    jitted = jax.jit(shard_map(_body, mesh=mesh, in_specs=in_specs,
                               out_specs=out_specs, check_rep=False), keep_unused=True)

    def run(in_maps):
        concat_in = [np.concatenate([np.asarray(in_maps[c][n]) for c in range(n_cores)], axis=0)
                     for n in in_names]
        concat_zeros = [np.zeros((n_cores * z.shape[0], *z.shape[1:]), z.dtype) for z in zero_outs]
        outs = jitted(*concat_in, *concat_zeros)
        import jax as _jax
        _jax.block_until_ready(outs)
        return [{name: np.asarray(outs[i]).reshape(n_cores, *out_avals[i].shape)[c]
                 for i, name in enumerate(out_names)} for c in range(n_cores)]
    return run


def _get_runner(causal: bool):
    key = bool(causal)
    if key not in _CACHE:
        nc = _build_program(key)
        _CACHE[key] = _make_runner(nc, H)
    return _CACHE[key]


def _prep_head(q2, k2, v2, causal):
    """q2,k2,v2: [N, D] fp32 for one head. Returns per-core input dict."""
    import ml_dtypes
    qT = np.concatenate([q2.T, np.ones((1, N), np.float32)], axis=0)  # [65, N]
    kT = np.concatenate([k2.T, np.ones((1, N), np.float32)], axis=0)
    # v with ones col 64 (denominator) and zero col 65 (f32r even-width pad)
    v3 = np.concatenate([v2, np.ones((N, 1), np.float32),
                         np.zeros((N, 1), np.float32)], axis=1)      # [N, 66]
    v_r = np.ascontiguousarray(
        v3.reshape(NJC, 128, 66).transpose(1, 0, 2).reshape(128, NJC * 66)
    ).astype(ml_dtypes.bfloat16)
    if causal:
        vc = np.cumsum(v2, axis=0, dtype=np.float64).astype(np.float32)
    else:
        vc = np.broadcast_to(v2.sum(axis=0, dtype=np.float64).astype(np.float32), (N, 64))
    vcum = np.ascontiguousarray(
        vc.reshape(NT, 128, 64).transpose(1, 0, 2).reshape(128, NT * 64))
    return {"qT": np.ascontiguousarray(qT), "kT": np.ascontiguousarray(kT),
            "vr": v_r, "vcum": vcum}


def kernel(q, k, v, rpe_matrix, mask):
    causal = bool(np.asarray(mask).item()) if not isinstance(mask, (int, bool)) else bool(mask)
    q = np.asarray(q, dtype=np.float32)
    k = np.asarray(k, dtype=np.float32)
    v = np.asarray(v, dtype=np.float32)
    rpe = np.asarray(rpe_matrix, dtype=np.float32)

    RPW = 2560 if causal else 4608
    if causal:
        # u in [0, N-1]: rpe_rev[u] = rpe[2N-2-u] -> rows 2N-2 .. N-1
        rpe_rev = rpe[N - 1:2 * N - 1][::-1]             # [N, 64]
    else:
        rpe_rev = rpe[::-1]                              # [2N-1, 64]
    rpeT = np.zeros((65, RPW), dtype=np.float32)
    rpeT[0:64, :rpe_rev.shape[0]] = rpe_rev.T            # row 64 stays zero

    a = np.arange(128, dtype=np.float32)[:, None]
    tt = np.arange(NT, dtype=np.float32)[None, :]
    iota = (128 * tt + a + 1.0) if causal else np.full((128, NT), float(N), np.float32)
    iota = np.ascontiguousarray(iota.astype(np.float32))

    run = _get_runner(causal)
    in_maps = []
    for h in range(H):
        m = _prep_head(q[0, h], k[0, h], v[0, h], causal)
        m["rpeT"] = rpeT
        m["iota"] = iota
        in_maps.append(m)
    results = run(in_maps)
    # o stored [128, NT*64] with o_store[a, 64t+d] = o[128t+a, d]
    outs = []
    for h in range(H):
        oh = results[h]["o"].reshape(128, NT, 64).transpose(1, 0, 2).reshape(N, 64)
        outs.append(oh)
    out = np.stack(outs)[None]  # [1, H, N, 64]
    return out.astype(np.float32)


if __name__ == "__main__":
    rng = np.random.default_rng(0)
    q = rng.standard_normal((B, H, N, D), dtype=np.float32)
    k = rng.standard_normal((B, H, N, D), dtype=np.float32)
    v = rng.standard_normal((B, H, N, D), dtype=np.float32)
    rpe = rng.standard_normal((2 * N - 1, D), dtype=np.float32)
    o = kernel(q, k, v, rpe, 1)
    print("out", o.shape, o.dtype, np.abs(o).mean())
